# revision 1
# baseline (speedup 1.0000x reference)
"""HRM dense-transformer kernel for 8 trn2 NeuronCores.

Sharding: data-parallel over batch (4) x sequence-parallel (2).
Core c handles batch b=c//2, token half h=c%2 (512 tokens).
Per block each core computes q/k/v for its own tokens, all-gathers
k^T and v (bf16) within its pair, then computes attention for its 512
queries over all 1024 keys. All activations are stored feature-major
([feature(part), token(free)]); scores are computed transposed [tk, tq]
so the softmax sum is a ones-matmul partition reduction and no
transposes are needed anywhere. V is computed token-major directly by
swapping matmul operands, with a ones column appended (M=65 matmul) so
the softmax denominator falls out of the PV matmul.
"""

import os
import sys

sys.path.insert(0, "/opt/trn_rl_repo")

import ml_dtypes
import numpy as np

import concourse.bass as bass
import concourse.mybir as mybir
import concourse.tile as tile
from concourse import bacc
from concourse.bass_utils import run_bass_kernel_spmd

F32 = mybir.dt.float32
F16 = mybir.dt.float16
BF16 = mybir.dt.bfloat16
AF = mybir.ActivationFunctionType
MUL = mybir.AluOpType.mult

B, S, D, NH, HD = 4, 1024, 1024, 16, 64
INTER = 2816
T = S // 2              # own tokens per core
DT = D // 128           # 8 d-tiles
IT = INTER // 128       # 22 inter tiles
VF = NH * (HD + 1)      # 1040, v_aug feature width
EPS = 1e-5
KN = T * S              # kT elems (own): 1024 x 512
VN = T * VF             # v_aug elems (own): 512 x 1040
RG = [[0, 1], [2, 3], [4, 5], [6, 7]]

N_LEVEL_CALLS = int(os.environ.get("HRM_LEVEL_CALLS", "6"))

_CACHE = {}


def _rope(nc, sp, ps, out_ap, cos, sin):
    """out = ps*cos + rotate_half(ps)*sin  (partition dim = 2 heads x 64)."""
    t1 = sp.tile([128, 512], F32, tag="rope1", name="rope1")
    t2 = sp.tile([128, 512], F32, tag="rope2", name="rope2")
    nc.vector.tensor_tensor(t1[:], ps[:], cos[:], MUL)
    nc.vector.tensor_tensor(t2[0:32, :], ps[32:64, :], sin[0:32, :], MUL)
    nc.vector.tensor_tensor(t2[32:64, :], ps[0:32, :], sin[32:64, :], MUL)
    nc.vector.tensor_tensor(t2[64:96, :], ps[96:128, :], sin[64:96, :], MUL)
    nc.vector.tensor_tensor(t2[96:128, :], ps[64:96, :], sin[96:128, :], MUL)
    nc.vector.tensor_add(out=out_ap, in0=t1[:], in1=t2[:])


def _rmsnorm(nc, sp, psum, h, hb, ones128, ones1f, eps_ap):
    ss = psum.tile([1, 512], F32, tag="ss", name="ss")
    for dt in range(DT):
        r2 = sp.tile([128, 512], F16, tag="r2", name="r2")
        nc.vector.tensor_tensor(r2[:], h[:, dt, :], h[:, dt, :], MUL)
        nc.tensor.matmul(ss[:], ones128[:], r2[:], start=(dt == 0), stop=(dt == DT - 1))
    s1 = sp.tile([1, 512], F32, tag="s1", name="s1")
    nc.scalar.activation(s1[:], ss[:], AF.Sqrt, bias=eps_ap, scale=1.0 / D)
    rstd = sp.tile([1, 512], F32, tag="rstd", name="rstd")
    nc.vector.reciprocal(rstd[:], s1[:])
    bc = _bcast(nc, sp, psum, rstd, ones1f)
    for dt in range(DT):
        nc.vector.tensor_tensor(h[:, dt, :], h[:, dt, :], bc[:], MUL)
        nc.vector.tensor_copy(out=hb[:, dt, :], in_=h[:, dt, :])


def _bcast(nc, sp, psum, row_f32, ones1f):
    """Broadcast [1,512] f32 across 128 partitions via K=1 matmul."""
    pb = psum.tile([128, 512], F32, tag="bc", bufs=2, name="pb")
    nc.tensor.matmul(pb[:], ones1f[:], row_f32[:], start=True, stop=True)
    bc = sp.tile([128, 512], F32, tag="bcsb", name="bc")
    nc.scalar.copy(bc[:], pb[:])
    return bc


def build_kernel():
    nc = bacc.Bacc("TRN2", target_bir_lowering=False, debug=False, num_devices=8)

    inp = {}
    for nm, shape, dt in [
        ("zL", [D, T], F32), ("zH", [D, T], F32), ("emb", [D, T], F32),
        ("cosT", [128, T], F32), ("sinT", [128, T], F32),
        ("L_wqT", [2, D, D], F16), ("L_wkT", [2, D, D], F16),
        ("L_wvT", [2, D, D], F16), ("L_woT", [2, D, D], F16),
        ("L_guT", [2, D, 2 * INTER], F16), ("L_dnT", [2, INTER, D], F16),
        ("H_wqT", [2, D, D], F16), ("H_wkT", [2, D, D], F16),
        ("H_wvT", [2, D, D], F16), ("H_woT", [2, D, D], F16),
        ("H_guT", [2, D, 2 * INTER], F16), ("H_dnT", [2, INTER, D], F16),
    ]:
        inp[nm] = nc.dram_tensor(nm, shape, dt, kind="ExternalInput")
    out_t = nc.dram_tensor("zH_out", [D, T], F32, kind="ExternalOutput")

    seq = os.environ.get("HRM_SEQ", "")
    if seq:
        level_calls = list(seq)
    else:
        level_calls = (["L", "L", "H"] * 2)[:N_LEVEL_CALLS]

    with tile.TileContext(nc) as tc:
        with (
            tc.tile_pool(name="state", bufs=1) as st,
            tc.tile_pool(name="sp", bufs=2) as sp,
            tc.tile_pool(name="big", bufs=2) as bigp,
            tc.tile_pool(name="w128", bufs=4) as w128p,
            tc.tile_pool(name="w256", bufs=2) as w256p,
            tc.tile_pool(name="wd", bufs=2) as wdp,
            tc.tile_pool(name="pt", bufs=2) as ptp,
            tc.tile_pool(name="psum", bufs=1, space="PSUM") as psum,
            tc.tile_pool(name="dram", bufs=2, space="DRAM") as dram,
        ):
            zL = st.tile([128, DT, T], F32, name="zL_sb")
            zH = st.tile([128, DT, T], F32, name="zH_sb")
            emb = st.tile([128, DT, T], F32, name="emb_sb")
            cos = st.tile([128, T], F32, name="cos_sb")
            sin = st.tile([128, T], F32, name="sin_sb")
            hb = st.tile([128, DT, T], F16, name="hb")
            qT = st.tile([128, DT, T], F16, name="qT")
            kst = st.tile([128, DT, T], F16, name="kst")
            vst = st.tile([128, 4, VF], BF16, name="vst")
            oT = st.tile([128, DT, T], F16, name="oT")
            ones128 = st.tile([128, 1], F16, name="ones128")
            ones1f = st.tile([1, 128], F32, name="ones1f")
            epsc = st.tile([1, 1], F32, name="epsc")

            nc.sync.dma_start(zL[:], inp["zL"].rearrange("(dt p) t -> p dt t", p=128))
            nc.sync.dma_start(zH[:], inp["zH"].rearrange("(dt p) t -> p dt t", p=128))
            nc.sync.dma_start(emb[:], inp["emb"].rearrange("(dt p) t -> p dt t", p=128))
            nc.sync.dma_start(cos[:], inp["cosT"][:])
            nc.sync.dma_start(sin[:], inp["sinT"][:])
            nc.vector.memset(ones128[:], 1.0)
            nc.vector.memset(ones1f[:], 1.0)
            nc.vector.memset(epsc[:], EPS)
            neg8 = st.tile([128, 1], F32, name="neg8")
            nc.vector.memset(neg8[:], -8.0)
            # ones columns of v_aug (written once; data copies avoid them)
            nc.vector.memset(
                vst.rearrange("p tt (h c) -> p tt h c", c=HD + 1)[:, :, :, HD : HD + 1],
                1.0,
            )

            def block(h, wq, wk, wv, wo, gu, dn):
                gin_k = dram.tile([KN], F16, tag="gin_k", name="gin_k")
                gout_k = dram.tile([2 * KN], F16, tag="gout_k", name="gout_k")
                gin_v = dram.tile([VN], BF16, tag="gin_v", name="gin_v")
                gout_v = dram.tile([2 * VN], BF16, tag="gout_v", name="gout_v")

                # ---- k projection + rope ----
                for ot in range(DT):
                    w = w128p.tile([128, DT, 128], F16, tag="w128", name="wk")
                    nc.sync.dma_start(w[:], wk[:, :, ot * 128 : (ot + 1) * 128])
                    ps = psum.tile([128, 512], F32, tag="mm", bufs=3, name="psk")
                    for dt in range(DT):
                        nc.tensor.matmul(ps[:], w[:, dt, :], hb[:, dt, :],
                                         start=(dt == 0), stop=(dt == DT - 1))
                    _rope(nc, sp, ps, kst[:, ot, :], cos, sin)
                # ---- send + gather k (overlaps v/q projection) ----
                nc.sync.dma_start(
                    gin_k[:].rearrange("(dt p t) -> p dt t", p=128, t=T), kst[:])
                nc.gpsimd.collective_compute(
                    "AllGather", mybir.AluOpType.bypass, replica_groups=RG,
                    ins=[gin_k.opt()], outs=[gout_k.opt()])
                # ---- v projection (token-major) ----
                vsr = vst.rearrange("p tt (hh c) -> p tt hh c", c=HD + 1)
                for oc in range(4):
                    w = w256p.tile([128, DT, 256], F16, tag="w256", name="wv")
                    nc.sync.dma_start(w[:], wv[:, :, oc * 256 : (oc + 1) * 256])
                    for tt in range(4):
                        ps = psum.tile([128, 512], F32, tag="mm", bufs=3, name="psv")[:, 0:256]
                        for dt in range(DT):
                            nc.tensor.matmul(
                                ps[:], hb[:, dt, tt * 128 : (tt + 1) * 128],
                                w[:, dt, :], start=(dt == 0), stop=(dt == DT - 1))
                        nc.vector.tensor_copy(
                            out=vsr[:, tt, oc * 4 : (oc + 1) * 4, 0:HD],
                            in_=ps.rearrange("p (hh c) -> p hh c", c=HD))
                # ---- send + gather v (k already in flight) ----
                nc.sync.dma_start(
                    gin_v[:].rearrange("(tt p f) -> p tt f", p=128, f=VF), vst[:])
                nc.gpsimd.collective_compute(
                    "AllGather", mybir.AluOpType.bypass, replica_groups=RG,
                    ins=[gin_v.opt()], outs=[gout_v.opt()])
                # ---- q projection + rope (overlaps gather) ----
                for ot in range(DT):
                    w = w128p.tile([128, DT, 128], F16, tag="w128", name="wq")
                    nc.sync.dma_start(w[:], wq[:, :, ot * 128 : (ot + 1) * 128])
                    ps = psum.tile([128, 512], F32, tag="mm", bufs=3, name="psq")
                    for dt in range(DT):
                        nc.tensor.matmul(ps[:], w[:, dt, :], hb[:, dt, :],
                                         start=(dt == 0), stop=(dt == DT - 1))
                    _rope(nc, sp, ps, qT[:, ot, :], cos, sin)
                # ---- load gathered k/v ----
                kTf = bigp.tile([128, DT, S], F16, tag="big", name="kTf")
                vf = bigp.tile([128, DT, VF], BF16, tag="big", name="vf")
                for r in range(2):
                    nc.sync.dma_start(
                        kTf[:, :, r * T : (r + 1) * T],
                        gout_k[r * KN : (r + 1) * KN].rearrange(
                            "(dt p t) -> p dt t", p=128, t=T))
                    nc.sync.dma_start(
                        vf[:, 4 * r : 4 * r + 4, :],
                        gout_v[r * VN : (r + 1) * VN].rearrange(
                            "(tt p f) -> p tt f", p=128, f=VF))
                # ---- attention, head pairs: scores A/B interleaved over
                # row groups (concurrent on PE), normalize deferred so PE
                # never waits on the DVE/ACT recip chain ----
                for ot in range(DT):
                    pts = []
                    pvs = []
                    for sub in range(2):
                        bp = sub * 64
                        pt = ptp.tile([128, DT, 512], BF16, tag="pt", bufs=2, name="pt")
                        pts.append(pt)
                    for kt in range(DT):
                        for sub in range(2):
                            bp = sub * 64
                            pss = psum.tile([128, 512], F32, tag="mm", bufs=3,
                                            name="pss")
                            nc.tensor.matmul(
                                pss[:],
                                kTf[bp : bp + 64, ot, kt * 128 : (kt + 1) * 128],
                                qT[bp : bp + 64, ot, :],
                                start=True, stop=True, tile_position=(bp, 0))
                            nc.scalar.activation(pts[sub][:, kt, :], pss[:],
                                                 AF.Exp, scale=0.125)
                    for sub in range(2):
                        hh = ot * 2 + sub
                        pv = psum.tile([128, 512], F32, tag="pv", bufs=2, name="pv")
                        for kt in range(DT):
                            nc.tensor.matmul(
                                pv[0 : HD + 1, :],
                                vf[:, kt, hh * (HD + 1) : (hh + 1) * (HD + 1)],
                                pts[sub][:, kt, :],
                                start=(kt == 0), stop=(kt == DT - 1))
                        pvs.append(pv)
                    for sub in range(2):
                        hh = ot * 2 + sub
                        bp = sub * 64
                        pv = pvs[sub]
                        recip = sp.tile([1, 512], F32, tag="recip", name="recip")
                        nc.vector.reciprocal(recip[:], pv[HD : HD + 1, :])
                        bc = _bcast(nc, sp, psum, recip, ones1f)
                        nc.vector.tensor_tensor(
                            oT[bp : bp + 64, ot, :], pv[0:HD, :], bc[0:HD, :], MUL)
                # ---- o projection + residual ----
                for dt2 in range(DT):
                    w = w128p.tile([128, DT, 128], F16, tag="w128", name="wo")
                    nc.sync.dma_start(w[:], wo[:, :, dt2 * 128 : (dt2 + 1) * 128])
                    ps = psum.tile([128, 512], F32, tag="mm", bufs=3, name="pso")
                    for et in range(DT):
                        nc.tensor.matmul(ps[:], w[:, et, :], oT[:, et, :],
                                         start=(et == 0), stop=(et == DT - 1))
                    nc.vector.tensor_add(out=h[:, dt2, :], in0=h[:, dt2, :], in1=ps[:])
                _rmsnorm(nc, sp, psum, h, hb, ones128, ones1f, epsc[:])
                # ---- MLP ----
                act = bigp.tile([128, IT, 512], F16, tag="big", name="act")
                for it in range(IT):
                    wg = w128p.tile([128, DT, 128], F16, tag="w128", name="wg")
                    wu = w128p.tile([128, DT, 128], F16, tag="w128", name="wu")
                    nc.sync.dma_start(wg[:], gu[:, :, it * 128 : (it + 1) * 128])
                    nc.sync.dma_start(
                        wu[:], gu[:, :, INTER + it * 128 : INTER + (it + 1) * 128])
                    psg = psum.tile([128, 512], F32, tag="mm", bufs=3, name="psg")
                    psu = psum.tile([128, 512], F32, tag="mm", bufs=3, name="psu")
                    for dt in range(DT):
                        nc.tensor.matmul(psg[:], wg[:, dt, :], hb[:, dt, :],
                                         start=(dt == 0), stop=(dt == DT - 1))
                    for dt in range(DT):
                        nc.tensor.matmul(psu[:], wu[:, dt, :], hb[:, dt, :],
                                         start=(dt == 0), stop=(dt == DT - 1))
                    sg = sp.tile([128, 512], F16, tag="sg", name="sg")
                    nc.scalar.activation(sg[:], psg[:], AF.Silu)
                    nc.vector.tensor_tensor(act[:, it, :], psu[:], sg[:], MUL)
                for dt2 in range(DT):
                    w = wdp.tile([128, IT, 128], F16, tag="wd", name="wdn")
                    nc.sync.dma_start(w[:], dn[:, :, dt2 * 128 : (dt2 + 1) * 128])
                    ps = psum.tile([128, 512], F32, tag="mm", bufs=3, name="psd")
                    for it in range(IT):
                        nc.tensor.matmul(ps[:], w[:, it, :], act[:, it, :],
                                         start=(it == 0), stop=(it == IT - 1))
                    nc.vector.tensor_add(out=h[:, dt2, :], in0=h[:, dt2, :], in1=ps[:])
                _rmsnorm(nc, sp, psum, h, hb, ones128, ones1f, epsc[:])

            def wrearr(ap):  # [K, M] -> [128, K//128, M] tiled view
                return ap.rearrange("(kt p) m -> p kt m", p=128)

            for lvl in level_calls:
                if lvl == "L":
                    h = zL
                    for dt in range(DT):
                        nc.vector.tensor_add(out=h[:, dt, :], in0=h[:, dt, :],
                                             in1=zH[:, dt, :])
                        nc.vector.tensor_add(out=h[:, dt, :], in0=h[:, dt, :],
                                             in1=emb[:, dt, :])
                    pre = "L"
                else:
                    h = zH
                    for dt in range(DT):
                        nc.vector.tensor_add(out=h[:, dt, :], in0=h[:, dt, :],
                                             in1=zL[:, dt, :])
                    pre = "H"
                for dt in range(DT):
                    nc.vector.tensor_copy(out=hb[:, dt, :], in_=h[:, dt, :])
                for i in range(2):
                    block(
                        h,
                        wrearr(inp[f"{pre}_wqT"][i]), wrearr(inp[f"{pre}_wkT"][i]),
                        wrearr(inp[f"{pre}_wvT"][i]), wrearr(inp[f"{pre}_woT"][i]),
                        wrearr(inp[f"{pre}_guT"][i]), wrearr(inp[f"{pre}_dnT"][i]),
                    )

            nc.sync.dma_start(
                out_t.rearrange("(dt p) t -> p dt t", p=128), zH[:])

    nc.compile()
    return nc


def _prep_weights(inputs):
    bf = np.float16
    w = {}
    for pre in ("L", "H"):
        for nm, src in [("wqT", "wq"), ("wkT", "wk"), ("wvT", "wv"), ("woT", "wo"),
                        ("guT", "gu"), ("dnT", "dn")]:
            a = np.asarray(inputs[f"{pre}_{src}"])
            w[f"{pre}_{nm}"] = np.ascontiguousarray(
                a.transpose(0, 2, 1)).astype(bf)
    cos = np.asarray(inputs["cos"])  # [S, 64]
    sin = np.asarray(inputs["sin"])
    cosT = np.tile(cos.T, (2, 1)).astype(np.float32)          # [128, S]
    sinT_s = sin.T.copy()
    sinT_s[:32] *= -1.0
    sinT = np.tile(sinT_s, (2, 1)).astype(np.float32)          # [128, S]
    return w, cosT, sinT


def kernel(**inputs):
    key = "nc"
    if key not in _CACHE:
        _CACHE[key] = build_kernel()
    nc = _CACHE[key]

    w, cosT, sinT = _prep_weights(inputs)
    zL = np.asarray(inputs["z_L"], np.float32)
    zH = np.asarray(inputs["z_H"], np.float32)
    emb = np.asarray(inputs["input_emb"], np.float32)

    in_maps = []
    for c in range(8):
        b, half = c // 2, c % 2
        sl = slice(half * T, (half + 1) * T)
        m = {
            "zL": np.ascontiguousarray(zL[b].T[:, sl]),
            "zH": np.ascontiguousarray(zH[b].T[:, sl]),
            "emb": np.ascontiguousarray(emb[b].T[:, sl]),
            "cosT": np.ascontiguousarray(cosT[:, sl]),
            "sinT": np.ascontiguousarray(sinT[:, sl]),
        }
        m.update(w)
        in_maps.append(m)

    trace = os.environ.get("HRM_TRACE", "0") == "1"
    res = run_bass_kernel_spmd(nc, in_maps, core_ids=list(range(8)), trace=trace)
    _CACHE["last_result"] = res

    out = np.empty((B, S, D), np.float32)
    for c in range(8):
        b, half = c // 2, c % 2
        out[b, half * T : (half + 1) * T, :] = res.results[c]["zH_out"].T
    return out


if __name__ == "__main__":
    rng = np.random.default_rng(0)
    ins = {
        "z_H": rng.standard_normal((B, S, D), np.float32),
        "z_L": rng.standard_normal((B, S, D), np.float32),
        "input_emb": rng.standard_normal((B, S, D), np.float32),
    }
    sd = 1.0 / np.sqrt(D)
    si = 1.0 / np.sqrt(INTER)
    for pre in ("L", "H"):
        for nm, shape, s in [("wq", (2, D, D), sd), ("wk", (2, D, D), sd),
                             ("wv", (2, D, D), sd), ("wo", (2, D, D), sd),
                             ("gu", (2, 2 * INTER, D), sd), ("dn", (2, D, INTER), si)]:
            ins[f"{pre}_{nm}"] = rng.standard_normal(shape, np.float32) * s
    inv = 1.0 / (10000.0 ** (np.arange(0, HD, 2, np.float32) / HD))
    fr = np.outer(np.arange(S, np.float32), inv)
    e = np.concatenate([fr, fr], -1)
    ins["cos"], ins["sin"] = np.cos(e).astype(np.float32), np.sin(e).astype(np.float32)
    out = kernel(**ins)
    print("out", out.shape, out.dtype, np.abs(out).mean())



# revision 10
# speedup vs baseline: 1.1684x; 1.1684x over previous
"""HRM dense-transformer kernel for 8 trn2 NeuronCores.

Sharding: data-parallel over batch (4) x sequence-parallel (2).
Core c handles batch b=c//2, token half h=c%2 (512 tokens).
Per block each core computes q/k/v for its own tokens, all-gathers
k^T and v (within its pair), then computes attention for its 512
queries over all 1024 keys. All activations are stored feature-major
([feature(part), token(free)]); scores are computed transposed [tk, tq]
so the softmax sum is a ones-matmul partition reduction. V is computed
token-major directly by swapping matmul operands, with a ones column
appended (M=65 matmul) so the softmax denominator falls out of the PV
matmul.

Perf structure (v2):
- scores are written into [128, 2, 512] 2-bank PSUM chunks and exp'd
  in [128, 1024] batches to amortize ACT per-op overhead.
- softmax normalize is deferred per head-pair: denominators collected
  to [2, 512], reciprocal via the fast custom-DVE approx, broadcast by
  ONE fp16 K=2 matmul (both heads at once) instead of fp32 K=1 pairs.
- rmsnorm uses ACT Square (in every act table -> no table reload) for
  h^2 and ACT Rsqrt for 1/sqrt(ms+eps); broadcast matmul in fp16.
- ACT table switches (Exp/Rsqrt/Silu) are prefetched with dummy [1,1]
  activations during PE-heavy phases.
- weights are pre-tiled host-side so every weight DMA reads 2KB
  contiguous per partition; gate+up column blocks are interleaved so
  one DMA feeds both.
"""

import os
import sys

sys.path.insert(0, "/opt/trn_rl_repo")

import ml_dtypes
import numpy as np

import concourse.bass as bass
import concourse.mybir as mybir
import concourse.tile as tile
from concourse import bacc
from concourse.bass_utils import run_bass_kernel_spmd

F32 = mybir.dt.float32
F16 = mybir.dt.float16
BF16 = mybir.dt.bfloat16
AF = mybir.ActivationFunctionType
MUL = mybir.AluOpType.mult

B, S, D, NH, HD = 4, 1024, 1024, 16, 64
INTER = 2816
T = S // 2              # own tokens per core
DT = D // 128           # 8 d-tiles
IT = INTER // 128       # 22 inter tiles
VF = NH * (HD + 1)      # 1040, v_aug feature width
EPS = 1e-5
KN = T * S              # kT elems (own): 1024 x 512
VN = T * VF             # v_aug elems (own): 512 x 1040
RG = [[0, 1], [2, 3], [4, 5], [6, 7]]

N_LEVEL_CALLS = int(os.environ.get("HRM_LEVEL_CALLS", "6"))
EXPSINGLE = os.environ.get("HRM_EXPSINGLE", "0") == "1"
NOPOOL = os.environ.get("HRM_NOPOOL", "0") == "1"
OLDNORM = os.environ.get("HRM_OLDNORM", "0") == "1"
OLDRMS = os.environ.get("HRM_OLDRMS", "0") == "1"

_CACHE = {}


def build_kernel():
    nc = bacc.Bacc("TRN2", target_bir_lowering=False, debug=False, num_devices=8)

    inp = {}
    for nm, shape, dt in [
        ("zL", [D, T], F32), ("zH", [D, T], F32), ("emb", [D, T], F32),
        ("cosT", [128, T], F32), ("sinT", [128, T], F32),
        # pre-tiled weights: [layer, out-tile, 128(p=in), in-tile, m]
        ("L_wqT", [2, DT, 128, DT, 128], F16),
        ("L_wkT", [2, DT, 128, DT, 128], F16),
        ("L_wvT", [2, 4, 128, DT, 256], F16),
        ("L_woT", [2, DT, 128, DT, 128], F16),
        ("L_guT", [2, IT, 128, DT, 256], F16),
        ("L_dnT", [2, DT, 128, IT, 128], F16),
        ("H_wqT", [2, DT, 128, DT, 128], F16),
        ("H_wkT", [2, DT, 128, DT, 128], F16),
        ("H_wvT", [2, 4, 128, DT, 256], F16),
        ("H_woT", [2, DT, 128, DT, 128], F16),
        ("H_guT", [2, IT, 128, DT, 256], F16),
        ("H_dnT", [2, DT, 128, IT, 128], F16),
    ]:
        inp[nm] = nc.dram_tensor(nm, shape, dt, kind="ExternalInput")
    out_t = nc.dram_tensor("zH_out", [D, T], F32, kind="ExternalOutput")

    seq = os.environ.get("HRM_SEQ", "")
    if seq:
        level_calls = list(seq)
    else:
        level_calls = (["L", "L", "H"] * 2)[:N_LEVEL_CALLS]

    with tile.TileContext(nc) as tc:
        with (
            tc.tile_pool(name="state", bufs=1) as st,
            tc.tile_pool(name="sp", bufs=2) as sp,
            tc.tile_pool(name="big", bufs=2) as bigp,
            tc.tile_pool(name="w128", bufs=4) as w128p,
            tc.tile_pool(name="w256", bufs=2) as w256p,
            tc.tile_pool(name="wd", bufs=2) as wdp,
            tc.tile_pool(name="pt", bufs=2) as ptp,
            tc.tile_pool(name="psum", bufs=1, space="PSUM") as psum,
            tc.tile_pool(name="dram", bufs=2, space="DRAM") as dram,
        ):
            zL = st.tile([128, DT, T], F32, name="zL_sb")
            zH = st.tile([128, DT, T], F32, name="zH_sb")
            emb = st.tile([128, DT, T], F32, name="emb_sb")
            cos = st.tile([128, T], F32, name="cos_sb")
            sin = st.tile([128, T], F32, name="sin_sb")
            hb = st.tile([128, DT, T], F16, name="hb")
            qT = st.tile([128, DT, T], F16, name="qT")
            kst = st.tile([128, DT, T], F16, name="kst")
            vst = st.tile([128, 4, VF], BF16, name="vst")
            oT = st.tile([128, DT, T], F16, name="oT")
            ones128 = st.tile([128, 1], F16, name="ones128")
            ones1f = st.tile([1, 128], F16, name="ones1f")
            sel64 = st.tile([64, 128], F16, name="sel64")
            den2 = st.tile([64, 512], F32, name="den2")
            denr = st.tile([64, 512], F32, name="denr")
            denr16 = st.tile([64, 512], F16, name="denr16")
            epsc = st.tile([1, 1], F32, name="epsc")
            dum = st.tile([1, 1], F32, name="dum")

            nc.sync.dma_start(zL[:], inp["zL"].rearrange("(dt p) t -> p dt t", p=128))
            nc.sync.dma_start(zH[:], inp["zH"].rearrange("(dt p) t -> p dt t", p=128))
            nc.sync.dma_start(emb[:], inp["emb"].rearrange("(dt p) t -> p dt t", p=128))
            nc.sync.dma_start(cos[:], inp["cosT"][:])
            nc.sync.dma_start(sin[:], inp["sinT"][:])
            nc.vector.memset(ones128[:], 1.0)
            nc.vector.memset(ones1f[:], 1.0)
            nc.vector.memset(epsc[:], EPS)
            nc.vector.memset(dum[:], 1.0)
            # selector for the softmax-denominator broadcast: row 0 feeds
            # out partitions 0-63 (even head), row 32 feeds 64-127 (odd).
            # den rows sit at partitions 0/32 (engine writes need 32-aligned
            # partition bases); unused rows stay at the 1.0 init so the
            # zero-weighted matmul columns never see NaN/Inf.
            nc.vector.memset(sel64[:], 0.0)
            nc.vector.memset(sel64[0:1, 0:64], 1.0)
            nc.vector.memset(sel64[32:33, 64:128], 1.0)
            nc.vector.memset(den2[:], 1.0)
            nc.vector.memset(denr[:], 1.0)
            nc.vector.memset(denr16[:], 1.0)
            # ones columns of v_aug (written once; data copies avoid them)
            nc.vector.memset(
                vst.rearrange("p tt (h c) -> p tt h c", c=HD + 1)[:, :, :, HD : HD + 1],
                1.0,
            )

            def preload(func):
                # dummy activation to pull the act table in early
                nc.scalar.activation(dum[:], dum[:], func)

            peng = nc.vector if NOPOOL else nc.gpsimd

            ones1f32 = st.tile([1, 128], F32, name="ones1f32")
            nc.vector.memset(ones1f32[:], 1.0)

            def _bcast_f32(row_f32):
                pb = psum.tile([128, 512], F32, tag="mm", bufs=2, name="pbf")
                nc.tensor.matmul(pb[:], ones1f32[:], row_f32, start=True,
                                 stop=True)
                bcf = sp.tile([128, 512], F32, tag="bcf", name="bcf")
                nc.vector.tensor_copy(out=bcf[:], in_=pb[:])
                return bcf

            def _rope(ps, out_ap):
                """out = ps*cos + rotate_half(ps)*sin.

                partition dim = 2 heads x 64. sinT rows are pre-negated for
                the first half of each head so a single multiply-add works.
                4 DVE ops: full-tile cos mult, two 2x32-partition sin mults
                (strided partition view pairs {0-31,64-95} <-> {32-63,96-127}),
                one add.
                """
                t1 = sp.tile([128, 512], F16, tag="rope1", name="rope1")
                t2 = sp.tile([128, 512], F16, tag="rope2", name="rope2")
                nc.vector.tensor_tensor(t1[:], ps[:], cos[:], MUL)
                nc.vector.tensor_tensor(t2[0:32, :], ps[32:64, :],
                                        sin[0:32, :], MUL)
                nc.vector.tensor_tensor(t2[32:64, :], ps[0:32, :],
                                        sin[32:64, :], MUL)
                nc.vector.tensor_tensor(t2[64:96, :], ps[96:128, :],
                                        sin[64:96, :], MUL)
                nc.vector.tensor_tensor(t2[96:128, :], ps[64:96, :],
                                        sin[96:128, :], MUL)
                nc.vector.tensor_add(out=out_ap, in0=t1[:], in1=t2[:])

            def _rmsnorm(h, hb):
                """h *= rsqrt(mean(h^2)+eps); hb = f16(h). ACT Square feeds a
                ones-matmul partition reduction; ACT Rsqrt (table prefetched)
                gives rstd in fp16; K=1 fp16 matmul broadcasts it."""
                sst = psum.tile([65, 512], F32, tag="pv", bufs=2, name="sst")
                ss = sst[0:1, :]
                for dt in range(DT):
                    r2 = sp.tile([128, 512], F16, tag="r2", name="r2")
                    nc.scalar.activation(r2[:], h[:, dt, :], AF.Square)
                    nc.tensor.matmul(ss, ones128[:], r2[:], start=(dt == 0),
                                     stop=(dt == DT - 1))
                sq = sp.tile([1, 512], F32, tag="sq", bufs=1, name="sq")
                nc.scalar.activation(sq[:], ss, AF.Sqrt, bias=epsc[:],
                                     scale=1.0 / D)
                if OLDRMS:
                    rstdf = sp.tile([1, 512], F32, tag="rstdf", bufs=1,
                                    name="rstdf")
                    nc.vector.reciprocal(rstdf[:], sq[:])
                    bcf = _bcast_f32(rstdf[:])
                    for dt in range(DT):
                        nc.vector.tensor_tensor(h[:, dt, :], h[:, dt, :],
                                                bcf[:], MUL)
                        nc.vector.tensor_copy(out=hb[:, dt, :],
                                              in_=h[:, dt, :])
                else:
                    rstdf = sp.tile([1, 512], F32, tag="rstdf", bufs=1,
                                    name="rstdf")
                    nc.vector.reciprocal_approx_fast(out=rstdf[:], in_=sq[:])
                    rstd = sp.tile([1, 512], F16, tag="rstd", bufs=1,
                                   name="rstd")
                    nc.vector.tensor_copy(out=rstd[:], in_=rstdf[:])
                    pb = psum.tile([128, 512], F32, tag="mm", bufs=2,
                                   name="pbn")
                    nc.tensor.matmul(pb[:], ones1f[:], rstd[:], start=True,
                                     stop=True)
                    bc = sp.tile([128, 512], F16, tag="bcn", name="bcn")
                    nc.vector.tensor_copy(out=bc[:], in_=pb[:])
                    for dt in range(DT):
                        nc.vector.tensor_tensor(hb[:, dt, :], h[:, dt, :],
                                                bc[:], MUL)
                        peng.tensor_tensor(h[:, dt, :], h[:, dt, :],
                                           bc[:], MUL)

            def block(h, wq, wk, wv, wo, gu, dn):
                gin_k = dram.tile([KN], F16, tag="gin_k", name="gin_k")
                gout_k = dram.tile([2 * KN], F16, tag="gout_k", name="gout_k")
                gin_v = dram.tile([VN], BF16, tag="gin_v", name="gin_v")
                gout_v = dram.tile([2 * VN], BF16, tag="gout_v", name="gout_v")

                # ---- k projection + rope ----
                for ot in range(DT):
                    w = w128p.tile([128, DT, 128], F16, tag="w128", name="wk")
                    nc.sync.dma_start(w[:], wk[ot])
                    ps = psum.tile([128, 512], F32, tag="mm", bufs=2, name="psk")
                    for dt in range(DT):
                        nc.tensor.matmul(ps[:], w[:, dt, :], hb[:, dt, :],
                                         start=(dt == 0), stop=(dt == DT - 1))
                    _rope(ps, kst[:, ot, :])
                # ---- send + gather k (overlaps v/q projection) ----
                nc.sync.dma_start(
                    gin_k[:].rearrange("(dt p t) -> p dt t", p=128, t=T), kst[:])
                nc.gpsimd.collective_compute(
                    "AllGather", mybir.AluOpType.bypass, replica_groups=RG,
                    ins=[gin_k.opt()], outs=[gout_k.opt()])
                # ---- v projection (token-major) ----
                vsr = vst.rearrange("p tt (hh c) -> p tt hh c", c=HD + 1)
                for oc in range(4):
                    w = w256p.tile([128, DT, 256], F16, tag="w256", name="wv")
                    nc.sync.dma_start(w[:], wv[oc])
                    for tt in range(4):
                        ps = psum.tile([128, 512], F32, tag="mm", bufs=2,
                                       name="psv")[:, 0:256]
                        for dt in range(DT):
                            nc.tensor.matmul(
                                ps[:], hb[:, dt, tt * 128 : (tt + 1) * 128],
                                w[:, dt, :], start=(dt == 0), stop=(dt == DT - 1))
                        nc.vector.tensor_copy(
                            out=vsr[:, tt, oc * 4 : (oc + 1) * 4, 0:HD],
                            in_=ps.rearrange("p (hh c) -> p hh c", c=HD))
                # ---- send + gather v (k already in flight) ----
                nc.sync.dma_start(
                    gin_v[:].rearrange("(tt p f) -> p tt f", p=128, f=VF), vst[:])
                nc.gpsimd.collective_compute(
                    "AllGather", mybir.AluOpType.bypass, replica_groups=RG,
                    ins=[gin_v.opt()], outs=[gout_v.opt()])
                # ---- q projection + rope (overlaps gather) ----
                for ot in range(DT):
                    w = w128p.tile([128, DT, 128], F16, tag="w128", name="wq")
                    nc.sync.dma_start(w[:], wq[ot])
                    ps = psum.tile([128, 512], F32, tag="mm", bufs=2, name="psq")
                    for dt in range(DT):
                        nc.tensor.matmul(ps[:], w[:, dt, :], hb[:, dt, :],
                                         start=(dt == 0), stop=(dt == DT - 1))
                    _rope(ps, qT[:, ot, :])
                # ---- load gathered k/v ----
                kTf = bigp.tile([128, DT, S], F16, tag="big", name="kTf")
                vf = bigp.tile([128, DT, VF], BF16, tag="big", name="vf")
                for r in range(2):
                    nc.sync.dma_start(
                        kTf[:, :, r * T : (r + 1) * T],
                        gout_k[r * KN : (r + 1) * KN].rearrange(
                            "(dt p t) -> p dt t", p=128, t=T))
                    nc.sync.dma_start(
                        vf[:, 4 * r : 4 * r + 4, :],
                        gout_v[r * VN : (r + 1) * VN].rearrange(
                            "(tt p f) -> p tt f", p=128, f=VF))
                # ---- attention ----
                # scores into [128,2,512] psum chunks, exp'd in [128,1024]
                # batches; PV accumulates per sub; normalize deferred per ot
                # with fast-recip + one fp16 K=2 broadcast matmul.
                for ot in range(DT):
                    pts = []
                    for sub in range(2):
                        bp = sub * 64
                        pt = ptp.tile([128, DT, 512], BF16, tag="pt", bufs=2,
                                      name="pt")
                        pts.append(pt)
                        for kc in range(4):
                            sc = psum.tile([128, 2, 512], F32, tag="sc", bufs=2,
                                           name="sc")
                            for j in range(2):
                                kt = kc * 2 + j
                                nc.tensor.matmul(
                                    sc[:, j, :],
                                    kTf[bp : bp + 64, ot, kt * 128 : (kt + 1) * 128],
                                    qT[bp : bp + 64, ot, :],
                                    start=True, stop=True, tile_position=(bp, 0))
                            if EXPSINGLE:
                                for j in range(2):
                                    nc.scalar.activation(
                                        pt[:, kc * 2 + j, :], sc[:, j, :],
                                        AF.Exp, scale=0.125)
                            else:
                                nc.scalar.activation(
                                    pt[:, kc * 2 : kc * 2 + 2, :], sc[:],
                                    AF.Exp, scale=0.125)
                    pvs = []
                    for sub in range(2):
                        hh = ot * 2 + sub
                        pv = psum.tile([65, 512], F32, tag="pv", bufs=2, name="pv")
                        for kt in range(DT):
                            nc.tensor.matmul(
                                pv[:],
                                vf[:, kt, hh * (HD + 1) : (hh + 1) * (HD + 1)],
                                pts[sub][:, kt, :],
                                start=(kt == 0), stop=(kt == DT - 1))
                        nc.vector.tensor_copy(out=den2[sub * 32 : sub * 32 + 1, :],
                                              in_=pv[64:65, :])
                        pvs.append(pv)
                    if ot == DT - 1:
                        preload(AF.Sqrt)
                    if OLDNORM:
                        for sub in range(2):
                            bp = sub * 64
                            recip = sp.tile([1, 512], F32, tag="recip",
                                            name="recip")
                            nc.vector.reciprocal(recip[:],
                                                 pvs[sub][64:65, :])
                            bcf = _bcast_f32(recip[:])
                            nc.vector.tensor_tensor(
                                oT[bp : bp + 64, ot, :], pvs[sub][0:HD, :],
                                bcf[bp : bp + 64, :], MUL)
                    else:
                        nc.vector.reciprocal_approx_fast(out=denr[:],
                                                         in_=den2[:])
                        nc.vector.tensor_copy(out=denr16[:], in_=denr[:])
                        pb = psum.tile([128, 512], F32, tag="mm", bufs=2,
                                       name="pbc")
                        nc.tensor.matmul(pb[:], sel64[:], denr16[:],
                                         start=True, stop=True)
                        bc = sp.tile([128, 512], F16, tag="bc", name="bc")
                        nc.vector.tensor_copy(out=bc[:], in_=pb[:])
                        for sub in range(2):
                            bp = sub * 64
                            nc.vector.tensor_tensor(
                                oT[bp : bp + 64, ot, :], pvs[sub][0:HD, :],
                                bc[bp : bp + 64, :], MUL)
                # ---- o projection + residual ----
                for dt2 in range(DT):
                    w = w128p.tile([128, DT, 128], F16, tag="w128", name="wo")
                    nc.sync.dma_start(w[:], wo[dt2])
                    ps = psum.tile([128, 512], F32, tag="mm", bufs=2, name="pso")
                    for et in range(DT):
                        nc.tensor.matmul(ps[:], w[:, et, :], oT[:, et, :],
                                         start=(et == 0), stop=(et == DT - 1))
                    nc.vector.tensor_add(out=h[:, dt2, :], in0=h[:, dt2, :],
                                         in1=ps[:])
                _rmsnorm(h, hb)
                preload(AF.Silu)
                # ---- MLP ----
                act = bigp.tile([128, IT, 512], F16, tag="big", name="act")
                for it in range(IT):
                    wgu = w256p.tile([128, DT, 256], F16, tag="w256", name="wgu")
                    nc.sync.dma_start(wgu[:], gu[it])
                    sc = psum.tile([128, 2, 512], F32, tag="sc", bufs=2,
                                   name="scm")
                    for dt in range(DT):
                        nc.tensor.matmul(sc[:, 0, :], wgu[:, dt, 0:128],
                                         hb[:, dt, :],
                                         start=(dt == 0), stop=(dt == DT - 1))
                    for dt in range(DT):
                        nc.tensor.matmul(sc[:, 1, :], wgu[:, dt, 128:256],
                                         hb[:, dt, :],
                                         start=(dt == 0), stop=(dt == DT - 1))
                    sg = sp.tile([128, 512], F16, tag="sg", name="sg")
                    nc.scalar.activation(sg[:], sc[:, 0, :], AF.Silu)
                    nc.vector.tensor_tensor(act[:, it, :], sc[:, 1, :], sg[:], MUL)
                    if it == IT - 1:
                        preload(AF.Sqrt)
                for dt2 in range(DT):
                    w = wdp.tile([128, IT, 128], F16, tag="wd", name="wdn")
                    nc.sync.dma_start(w[:], dn[dt2])
                    ps = psum.tile([128, 512], F32, tag="mm", bufs=2, name="psd")
                    for it in range(IT):
                        nc.tensor.matmul(ps[:], w[:, it, :], act[:, it, :],
                                         start=(it == 0), stop=(it == IT - 1))
                    nc.vector.tensor_add(out=h[:, dt2, :], in0=h[:, dt2, :],
                                         in1=ps[:])
                _rmsnorm(h, hb)
                preload(AF.Exp)

            for lvl in level_calls:
                if lvl == "L":
                    h = zL
                    for dt in range(DT):
                        peng.tensor_tensor(h[:, dt, :], h[:, dt, :],
                                           zH[:, dt, :],
                                           mybir.AluOpType.add)
                        peng.tensor_tensor(h[:, dt, :], h[:, dt, :],
                                           emb[:, dt, :],
                                           mybir.AluOpType.add)
                    pre = "L"
                else:
                    h = zH
                    for dt in range(DT):
                        peng.tensor_tensor(h[:, dt, :], h[:, dt, :],
                                           zL[:, dt, :],
                                           mybir.AluOpType.add)
                    pre = "H"
                for dt in range(DT):
                    nc.vector.tensor_copy(out=hb[:, dt, :], in_=h[:, dt, :])
                for i in range(2):
                    block(
                        h,
                        inp[f"{pre}_wqT"][i], inp[f"{pre}_wkT"][i],
                        inp[f"{pre}_wvT"][i], inp[f"{pre}_woT"][i],
                        inp[f"{pre}_guT"][i], inp[f"{pre}_dnT"][i],
                    )

            nc.sync.dma_start(
                out_t.rearrange("(dt p) t -> p dt t", p=128), zH[:])

    nc.compile()
    return nc


def _prep_weights(inputs):
    bf = np.float16
    w = {}
    for pre in ("L", "H"):
        # [out, in] torch-style weights -> pre-tiled [L, ot, p(in), dt(in), m]
        for nm, src, mtile in [("wqT", "wq", 128), ("wkT", "wk", 128),
                               ("woT", "wo", 128)]:
            a = np.asarray(inputs[f"{pre}_{src}"])  # [2, D, D] = [l, o, i]
            t = a.reshape(2, DT, 128, DT, 128)       # [l, ot, m, dt, p]
            w[f"{pre}_{nm}"] = np.ascontiguousarray(
                t.transpose(0, 1, 4, 3, 2)).astype(bf)
        a = np.asarray(inputs[f"{pre}_wv"])          # [2, D, D]
        t = a.reshape(2, 4, 256, DT, 128)            # [l, oc, m, dt, p]
        w[f"{pre}_wvT"] = np.ascontiguousarray(
            t.transpose(0, 1, 4, 3, 2)).astype(bf)
        g = np.asarray(inputs[f"{pre}_gu"])          # [2, 2*INTER, D]
        gate = g[:, :INTER].reshape(2, IT, 128, DT, 128)
        up = g[:, INTER:].reshape(2, IT, 128, DT, 128)
        gu = np.concatenate([gate, up], axis=2)      # [l, it, 256(m), dt, p]
        w[f"{pre}_guT"] = np.ascontiguousarray(
            gu.transpose(0, 1, 4, 3, 2)).astype(bf)  # [l, it, p, dt, 256]
        d = np.asarray(inputs[f"{pre}_dn"])          # [2, D, INTER]
        t = d.reshape(2, DT, 128, IT, 128)           # [l, ot, m, it, p]
        w[f"{pre}_dnT"] = np.ascontiguousarray(
            t.transpose(0, 1, 4, 3, 2)).astype(bf)   # [l, ot, p, it, 128]
    cos = np.asarray(inputs["cos"])  # [S, 64]
    sin = np.asarray(inputs["sin"])
    cosT = np.tile(cos.T, (2, 1)).astype(np.float32)          # [128, S]
    sinT_s = sin.T.copy()
    sinT_s[:32] *= -1.0
    sinT = np.tile(sinT_s, (2, 1)).astype(np.float32)          # [128, S]
    return w, cosT, sinT


def kernel(**inputs):
    key = "nc"
    if key not in _CACHE:
        _CACHE[key] = build_kernel()
    nc = _CACHE[key]

    w, cosT, sinT = _prep_weights(inputs)
    zL = np.asarray(inputs["z_L"], np.float32)
    zH = np.asarray(inputs["z_H"], np.float32)
    emb = np.asarray(inputs["input_emb"], np.float32)

    in_maps = []
    for c in range(8):
        b, half = c // 2, c % 2
        sl = slice(half * T, (half + 1) * T)
        m = {
            "zL": np.ascontiguousarray(zL[b].T[:, sl]),
            "zH": np.ascontiguousarray(zH[b].T[:, sl]),
            "emb": np.ascontiguousarray(emb[b].T[:, sl]),
            "cosT": np.ascontiguousarray(cosT[:, sl]),
            "sinT": np.ascontiguousarray(sinT[:, sl]),
        }
        m.update(w)
        in_maps.append(m)

    trace = os.environ.get("HRM_TRACE", "0") == "1"
    res = run_bass_kernel_spmd(nc, in_maps, core_ids=list(range(8)), trace=trace)
    _CACHE["last_result"] = res

    out = np.empty((B, S, D), np.float32)
    for c in range(8):
        b, half = c // 2, c % 2
        out[b, half * T : (half + 1) * T, :] = res.results[c]["zH_out"].T
    return out


if __name__ == "__main__":
    rng = np.random.default_rng(0)
    ins = {
        "z_H": rng.standard_normal((B, S, D), np.float32),
        "z_L": rng.standard_normal((B, S, D), np.float32),
        "input_emb": rng.standard_normal((B, S, D), np.float32),
    }
    sd = 1.0 / np.sqrt(D)
    si = 1.0 / np.sqrt(INTER)
    for pre in ("L", "H"):
        for nm, shape, s in [("wq", (2, D, D), sd), ("wk", (2, D, D), sd),
                             ("wv", (2, D, D), sd), ("wo", (2, D, D), sd),
                             ("gu", (2, 2 * INTER, D), sd), ("dn", (2, D, INTER), si)]:
            ins[f"{pre}_{nm}"] = rng.standard_normal(shape, np.float32) * s
    inv = 1.0 / (10000.0 ** (np.arange(0, HD, 2, np.float32) / HD))
    fr = np.outer(np.arange(S, np.float32), inv)
    e = np.concatenate([fr, fr], -1)
    ins["cos"], ins["sin"] = np.cos(e).astype(np.float32), np.sin(e).astype(np.float32)
    out = kernel(**ins)
    print("out", out.shape, out.dtype, np.abs(out).mean())


# revision 11
# speedup vs baseline: 1.1825x; 1.0121x over previous
"""HRM dense-transformer kernel for 8 trn2 NeuronCores.

Sharding: data-parallel over batch (4) x sequence-parallel (2).
Core c handles batch b=c//2, token half h=c%2 (512 tokens).
Per block each core computes q/k/v for its own tokens, all-gathers
k^T and v (within its pair), then computes attention for its 512
queries over all 1024 keys. All activations are stored feature-major
([feature(part), token(free)]); scores are computed transposed [tk, tq]
so the softmax sum is a ones-matmul partition reduction. V is computed
token-major directly by swapping matmul operands, with a ones column
appended (M=65 matmul) so the softmax denominator falls out of the PV
matmul.

Perf structure (v2):
- scores are written into [128, 2, 512] 2-bank PSUM chunks and exp'd
  in [128, 1024] batches to amortize ACT per-op overhead.
- softmax normalize is deferred per head-pair: denominators collected
  to [2, 512], reciprocal via the fast custom-DVE approx, broadcast by
  ONE fp16 K=2 matmul (both heads at once) instead of fp32 K=1 pairs.
- rmsnorm uses ACT Square (in every act table -> no table reload) for
  h^2 and ACT Rsqrt for 1/sqrt(ms+eps); broadcast matmul in fp16.
- ACT table switches (Exp/Rsqrt/Silu) are prefetched with dummy [1,1]
  activations during PE-heavy phases.
- weights are pre-tiled host-side so every weight DMA reads 2KB
  contiguous per partition; gate+up column blocks are interleaved so
  one DMA feeds both.
"""

import os
import sys

sys.path.insert(0, "/opt/trn_rl_repo")

import ml_dtypes
import numpy as np

import concourse.bass as bass
import concourse.mybir as mybir
import concourse.tile as tile
from concourse import bacc
from concourse.bass_utils import run_bass_kernel_spmd

F32 = mybir.dt.float32
F16 = mybir.dt.float16
BF16 = mybir.dt.bfloat16
AF = mybir.ActivationFunctionType
MUL = mybir.AluOpType.mult

B, S, D, NH, HD = 4, 1024, 1024, 16, 64
INTER = 2816
T = S // 2              # own tokens per core
DT = D // 128           # 8 d-tiles
IT = INTER // 128       # 22 inter tiles
VF = NH * (HD + 1)      # 1040, v_aug feature width
EPS = 1e-5
KN = T * S              # kT elems (own): 1024 x 512
VN = T * VF             # v_aug elems (own): 512 x 1040
RG = [[0, 1], [2, 3], [4, 5], [6, 7]]

N_LEVEL_CALLS = int(os.environ.get("HRM_LEVEL_CALLS", "6"))
EXPSINGLE = os.environ.get("HRM_EXPSINGLE", "0") == "1"
NOPOOL = os.environ.get("HRM_NOPOOL", "0") == "1"
OLDNORM = os.environ.get("HRM_OLDNORM", "0") == "1"
OLDRMS = os.environ.get("HRM_OLDRMS", "0") == "1"

_CACHE = {}


def build_kernel():
    nc = bacc.Bacc("TRN2", target_bir_lowering=False, debug=False, num_devices=8)

    inp = {}
    for nm, shape, dt in [
        ("zL", [D, T], F32), ("zH", [D, T], F32), ("emb", [D, T], F32),
        ("cosT", [128, T], F32), ("sinT", [128, T], F32),
        # pre-tiled weights: [layer, out-tile, 128(p=in), in-tile, m]
        ("L_wqT", [2, DT, 128, DT, 128], F16),
        ("L_wkT", [2, DT, 128, DT, 128], F16),
        ("L_wvT", [2, 4, 128, DT, 256], F16),
        ("L_woT", [2, DT, 128, DT, 128], F16),
        ("L_guT", [2, IT, 128, DT, 256], F16),
        ("L_dnT", [2, DT, 128, IT, 128], F16),
        ("H_wqT", [2, DT, 128, DT, 128], F16),
        ("H_wkT", [2, DT, 128, DT, 128], F16),
        ("H_wvT", [2, 4, 128, DT, 256], F16),
        ("H_woT", [2, DT, 128, DT, 128], F16),
        ("H_guT", [2, IT, 128, DT, 256], F16),
        ("H_dnT", [2, DT, 128, IT, 128], F16),
    ]:
        inp[nm] = nc.dram_tensor(nm, shape, dt, kind="ExternalInput")
    out_t = nc.dram_tensor("zH_out", [D, T], F32, kind="ExternalOutput")

    seq = os.environ.get("HRM_SEQ", "")
    if seq:
        level_calls = list(seq)
    else:
        level_calls = (["L", "L", "H"] * 2)[:N_LEVEL_CALLS]

    with tile.TileContext(nc) as tc:
        with (
            tc.tile_pool(name="state", bufs=1) as st,
            tc.tile_pool(name="sp", bufs=2) as sp,
            tc.tile_pool(name="big", bufs=2) as bigp,
            tc.tile_pool(name="w128", bufs=4) as w128p,
            tc.tile_pool(name="w256", bufs=2) as w256p,
            tc.tile_pool(name="wd", bufs=2) as wdp,
            tc.tile_pool(name="pt", bufs=2) as ptp,
            tc.tile_pool(name="psum", bufs=1, space="PSUM") as psum,
            tc.tile_pool(name="dram", bufs=2, space="DRAM") as dram,
        ):
            zL = st.tile([128, DT, T], F32, name="zL_sb")
            zH = st.tile([128, DT, T], F32, name="zH_sb")
            emb = st.tile([128, DT, T], F32, name="emb_sb")
            cos = st.tile([128, T], F32, name="cos_sb")
            sin = st.tile([128, T], F32, name="sin_sb")
            hb = st.tile([128, DT, T], F16, name="hb")
            qT = st.tile([128, DT, T], F16, name="qT")
            kst = st.tile([128, DT, T], F16, name="kst")
            vst = st.tile([128, 4, VF], BF16, name="vst")
            oT = st.tile([128, DT, T], F16, name="oT")
            ones128 = st.tile([128, 1], F16, name="ones128")
            ones1f = st.tile([1, 128], F16, name="ones1f")
            sel64 = st.tile([64, 128], F16, name="sel64")
            den2 = st.tile([64, 512], F32, name="den2")
            denr = st.tile([64, 512], F32, name="denr")
            denr16 = st.tile([64, 512], F16, name="denr16")
            epsc = st.tile([1, 1], F32, name="epsc")
            dum = st.tile([1, 1], F32, name="dum")

            nc.sync.dma_start(zL[:], inp["zL"].rearrange("(dt p) t -> p dt t", p=128))
            nc.sync.dma_start(zH[:], inp["zH"].rearrange("(dt p) t -> p dt t", p=128))
            nc.sync.dma_start(emb[:], inp["emb"].rearrange("(dt p) t -> p dt t", p=128))
            nc.sync.dma_start(cos[:], inp["cosT"][:])
            nc.sync.dma_start(sin[:], inp["sinT"][:])
            nc.vector.memset(ones128[:], 1.0)
            nc.vector.memset(ones1f[:], 1.0)
            nc.vector.memset(epsc[:], EPS)
            nc.vector.memset(dum[:], 1.0)
            # selector for the softmax-denominator broadcast: row 0 feeds
            # out partitions 0-63 (even head), row 32 feeds 64-127 (odd).
            # den rows sit at partitions 0/32 (engine writes need 32-aligned
            # partition bases); unused rows stay at the 1.0 init so the
            # zero-weighted matmul columns never see NaN/Inf.
            nc.vector.memset(sel64[:], 0.0)
            nc.vector.memset(sel64[0:1, 0:64], 1.0)
            nc.vector.memset(sel64[32:33, 64:128], 1.0)
            nc.vector.memset(den2[:], 1.0)
            nc.vector.memset(denr[:], 1.0)
            nc.vector.memset(denr16[:], 1.0)
            # ones columns of v_aug (written once; data copies avoid them)
            nc.vector.memset(
                vst.rearrange("p tt (h c) -> p tt h c", c=HD + 1)[:, :, :, HD : HD + 1],
                1.0,
            )

            def preload(func):
                # dummy activation to pull the act table in early
                nc.scalar.activation(dum[:], dum[:], func)

            peng = nc.vector if NOPOOL else nc.gpsimd

            ones1f32 = st.tile([1, 128], F32, name="ones1f32")
            nc.vector.memset(ones1f32[:], 1.0)

            def _bcast_f32(row_f32):
                pb = psum.tile([128, 512], F32, tag="mm", bufs=2, name="pbf")
                nc.tensor.matmul(pb[:], ones1f32[:], row_f32, start=True,
                                 stop=True)
                bcf = sp.tile([128, 512], F32, tag="bcf", bufs=1, name="bcf")
                nc.vector.tensor_copy(out=bcf[:], in_=pb[:])
                return bcf

            def _rope(ps, out_ap):
                """out = ps*cos + rotate_half(ps)*sin.

                partition dim = 2 heads x 64. sinT rows are pre-negated for
                the first half of each head so a single multiply-add works.
                4 DVE ops: full-tile cos mult, two 2x32-partition sin mults
                (strided partition view pairs {0-31,64-95} <-> {32-63,96-127}),
                one add.
                """
                t1 = sp.tile([128, 512], F16, tag="rope1", name="rope1")
                t2 = sp.tile([128, 512], F16, tag="rope2", name="rope2")
                nc.vector.tensor_tensor(t1[:], ps[:], cos[:], MUL)
                nc.vector.tensor_tensor(t2[0:32, :], ps[32:64, :],
                                        sin[0:32, :], MUL)
                nc.vector.tensor_tensor(t2[32:64, :], ps[0:32, :],
                                        sin[32:64, :], MUL)
                nc.vector.tensor_tensor(t2[64:96, :], ps[96:128, :],
                                        sin[64:96, :], MUL)
                nc.vector.tensor_tensor(t2[96:128, :], ps[64:96, :],
                                        sin[96:128, :], MUL)
                nc.vector.tensor_add(out=out_ap, in0=t1[:], in1=t2[:])

            def _rmsnorm(h, hb):
                """h *= rsqrt(mean(h^2)+eps); hb = f16(h). ACT Square feeds a
                ones-matmul partition reduction; ACT Rsqrt (table prefetched)
                gives rstd in fp16; K=1 fp16 matmul broadcasts it."""
                sst = psum.tile([65, 512], F32, tag="pv", bufs=2, name="sst")
                ss = sst[0:1, :]
                for dt in range(DT):
                    r2 = sp.tile([128, 512], F16, tag="r2", name="r2")
                    nc.scalar.activation(r2[:], h[:, dt, :], AF.Square)
                    nc.tensor.matmul(ss, ones128[:], r2[:], start=(dt == 0),
                                     stop=(dt == DT - 1))
                sq = sp.tile([1, 512], F32, tag="sq", bufs=1, name="sq")
                nc.scalar.activation(sq[:], ss, AF.Sqrt, bias=epsc[:],
                                     scale=1.0 / D)
                if OLDRMS:
                    rstdf = sp.tile([1, 512], F32, tag="rstdf", bufs=1,
                                    name="rstdf")
                    nc.vector.reciprocal(rstdf[:], sq[:])
                    bcf = _bcast_f32(rstdf[:])
                    for dt in range(DT):
                        nc.vector.tensor_tensor(h[:, dt, :], h[:, dt, :],
                                                bcf[:], MUL)
                        nc.vector.tensor_copy(out=hb[:, dt, :],
                                              in_=h[:, dt, :])
                else:
                    rstdf = sp.tile([1, 512], F32, tag="rstdf", bufs=1,
                                    name="rstdf")
                    nc.vector.reciprocal_approx_fast(out=rstdf[:], in_=sq[:])
                    rstd = sp.tile([1, 512], F16, tag="rstd", bufs=1,
                                   name="rstd")
                    nc.vector.tensor_copy(out=rstd[:], in_=rstdf[:])
                    pb = psum.tile([128, 512], F32, tag="mm", bufs=2,
                                   name="pbn")
                    nc.tensor.matmul(pb[:], ones1f[:], rstd[:], start=True,
                                     stop=True)
                    bc = sp.tile([128, 512], F16, tag="bcn", name="bcn")
                    nc.vector.tensor_copy(out=bc[:], in_=pb[:])
                    for dt in range(DT):
                        nc.vector.tensor_tensor(hb[:, dt, :], h[:, dt, :],
                                                bc[:], MUL)
                        peng.tensor_tensor(h[:, dt, :], h[:, dt, :],
                                           bc[:], MUL)

            def block(h, wq, wk, wv, wo, gu, dn):
                gin_k = dram.tile([KN], F16, tag="gin_k", name="gin_k")
                gout_k = dram.tile([2 * KN], F16, tag="gout_k", name="gout_k")
                gin_v = dram.tile([VN], BF16, tag="gin_v", name="gin_v")
                gout_v = dram.tile([2 * VN], BF16, tag="gout_v", name="gout_v")

                # ---- k projection + rope ----
                for ot in range(DT):
                    w = w128p.tile([128, DT, 128], F16, tag="w128", name="wk")
                    nc.sync.dma_start(w[:], wk[ot])
                    ps = psum.tile([128, 512], F32, tag="mm", bufs=2, name="psk")
                    for dt in range(DT):
                        nc.tensor.matmul(ps[:], w[:, dt, :], hb[:, dt, :],
                                         start=(dt == 0), stop=(dt == DT - 1))
                    _rope(ps, kst[:, ot, :])
                # ---- send + gather k (overlaps v/q projection) ----
                nc.sync.dma_start(
                    gin_k[:].rearrange("(dt p t) -> p dt t", p=128, t=T), kst[:])
                nc.gpsimd.collective_compute(
                    "AllGather", mybir.AluOpType.bypass, replica_groups=RG,
                    ins=[gin_k.opt()], outs=[gout_k.opt()])
                # ---- v projection (token-major) ----
                vsr = vst.rearrange("p tt (hh c) -> p tt hh c", c=HD + 1)
                for oc in range(4):
                    w = w256p.tile([128, DT, 256], F16, tag="w256", name="wv")
                    nc.sync.dma_start(w[:], wv[oc])
                    for tt in range(4):
                        ps = psum.tile([128, 512], F32, tag="mm", bufs=2,
                                       name="psv")[:, 0:256]
                        for dt in range(DT):
                            nc.tensor.matmul(
                                ps[:], hb[:, dt, tt * 128 : (tt + 1) * 128],
                                w[:, dt, :], start=(dt == 0), stop=(dt == DT - 1))
                        nc.vector.tensor_copy(
                            out=vsr[:, tt, oc * 4 : (oc + 1) * 4, 0:HD],
                            in_=ps.rearrange("p (hh c) -> p hh c", c=HD))
                # ---- send + gather v (k already in flight) ----
                nc.sync.dma_start(
                    gin_v[:].rearrange("(tt p f) -> p tt f", p=128, f=VF), vst[:])
                nc.gpsimd.collective_compute(
                    "AllGather", mybir.AluOpType.bypass, replica_groups=RG,
                    ins=[gin_v.opt()], outs=[gout_v.opt()])
                # ---- q projection + rope (overlaps gather) ----
                for ot in range(DT):
                    w = w128p.tile([128, DT, 128], F16, tag="w128", name="wq")
                    nc.sync.dma_start(w[:], wq[ot])
                    ps = psum.tile([128, 512], F32, tag="mm", bufs=2, name="psq")
                    for dt in range(DT):
                        nc.tensor.matmul(ps[:], w[:, dt, :], hb[:, dt, :],
                                         start=(dt == 0), stop=(dt == DT - 1))
                    _rope(ps, qT[:, ot, :])
                # ---- load gathered k/v ----
                kTf = bigp.tile([128, DT, S], F16, tag="big", name="kTf")
                vf = bigp.tile([128, DT, VF], BF16, tag="big", name="vf")
                for r in range(2):
                    nc.sync.dma_start(
                        kTf[:, :, r * T : (r + 1) * T],
                        gout_k[r * KN : (r + 1) * KN].rearrange(
                            "(dt p t) -> p dt t", p=128, t=T))
                    nc.sync.dma_start(
                        vf[:, 4 * r : 4 * r + 4, :],
                        gout_v[r * VN : (r + 1) * VN].rearrange(
                            "(tt p f) -> p tt f", p=128, f=VF))
                # ---- attention ----
                # scores into [128,2,512] psum chunks, exp'd in [128,1024]
                # batches; PV accumulates per sub; normalize deferred per ot
                # with fast-recip + one fp16 K=2 broadcast matmul.
                for ot in range(DT):
                    pts = []
                    for sub in range(2):
                        bp = sub * 64
                        pt = ptp.tile([128, DT, 512], BF16, tag="pt", bufs=2,
                                      name="pt")
                        pts.append(pt)
                        for kc in range(4):
                            sc = psum.tile([128, 2, 512], F32, tag="sc", bufs=2,
                                           name="sc")
                            for j in range(2):
                                kt = kc * 2 + j
                                nc.tensor.matmul(
                                    sc[:, j, :],
                                    kTf[bp : bp + 64, ot, kt * 128 : (kt + 1) * 128],
                                    qT[bp : bp + 64, ot, :],
                                    start=True, stop=True, tile_position=(bp, 0))
                            if EXPSINGLE:
                                for j in range(2):
                                    nc.scalar.activation(
                                        pt[:, kc * 2 + j, :], sc[:, j, :],
                                        AF.Exp, scale=0.125)
                            else:
                                nc.scalar.activation(
                                    pt[:, kc * 2 : kc * 2 + 2, :], sc[:],
                                    AF.Exp, scale=0.125)
                    pvs = []
                    for sub in range(2):
                        hh = ot * 2 + sub
                        pv = psum.tile([65, 512], F32, tag="pv", bufs=2, name="pv")
                        for kt in range(DT):
                            nc.tensor.matmul(
                                pv[:],
                                vf[:, kt, hh * (HD + 1) : (hh + 1) * (HD + 1)],
                                pts[sub][:, kt, :],
                                start=(kt == 0), stop=(kt == DT - 1))
                        nc.vector.tensor_copy(out=den2[sub * 32 : sub * 32 + 1, :],
                                              in_=pv[64:65, :])
                        pvs.append(pv)
                    if ot == DT - 1:
                        preload(AF.Sqrt)
                    if OLDNORM:
                        for sub in range(2):
                            bp = sub * 64
                            recip = sp.tile([1, 512], F32, tag="recip",
                                            name="recip")
                            nc.vector.reciprocal(recip[:],
                                                 pvs[sub][64:65, :])
                            bcf = _bcast_f32(recip[:])
                            nc.vector.tensor_tensor(
                                oT[bp : bp + 64, ot, :], pvs[sub][0:HD, :],
                                bcf[bp : bp + 64, :], MUL)
                    else:
                        nc.vector.reciprocal_approx_fast(out=denr[:],
                                                         in_=den2[:])
                        nc.vector.tensor_copy(out=denr16[:], in_=denr[:])
                        pb = psum.tile([128, 512], F32, tag="mm", bufs=2,
                                       name="pbc")
                        nc.tensor.matmul(pb[:], sel64[:], denr16[:],
                                         start=True, stop=True)
                        bc = sp.tile([128, 512], F16, tag="bc", name="bc")
                        nc.vector.tensor_copy(out=bc[:], in_=pb[:])
                        for sub in range(2):
                            bp = sub * 64
                            nc.vector.tensor_tensor(
                                oT[bp : bp + 64, ot, :], pvs[sub][0:HD, :],
                                bc[bp : bp + 64, :], MUL)
                # ---- o projection + residual ----
                for dt2 in range(DT):
                    w = w128p.tile([128, DT, 128], F16, tag="w128", name="wo")
                    nc.sync.dma_start(w[:], wo[dt2])
                    ps = psum.tile([128, 512], F32, tag="mm", bufs=2, name="pso")
                    for et in range(DT):
                        nc.tensor.matmul(ps[:], w[:, et, :], oT[:, et, :],
                                         start=(et == 0), stop=(et == DT - 1))
                    nc.vector.tensor_add(out=h[:, dt2, :], in0=h[:, dt2, :],
                                         in1=ps[:])
                _rmsnorm(h, hb)
                preload(AF.Silu)
                # ---- MLP ----
                act = bigp.tile([128, IT, 512], F16, tag="big", name="act")
                for it in range(IT):
                    wgu = w256p.tile([128, DT, 256], F16, tag="w256", name="wgu")
                    nc.sync.dma_start(wgu[:], gu[it])
                    sc = psum.tile([128, 2, 512], F32, tag="sc", bufs=2,
                                   name="scm")
                    for dt in range(DT):
                        nc.tensor.matmul(sc[:, 0, :], wgu[:, dt, 0:128],
                                         hb[:, dt, :],
                                         start=(dt == 0), stop=(dt == DT - 1))
                    for dt in range(DT):
                        nc.tensor.matmul(sc[:, 1, :], wgu[:, dt, 128:256],
                                         hb[:, dt, :],
                                         start=(dt == 0), stop=(dt == DT - 1))
                    sg = sp.tile([128, 512], F16, tag="sg", name="sg")
                    nc.scalar.activation(sg[:], sc[:, 0, :], AF.Silu)
                    nc.vector.tensor_tensor(act[:, it, :], sc[:, 1, :], sg[:], MUL)
                    if it == IT - 1:
                        preload(AF.Sqrt)
                for dt2 in range(DT):
                    w = wdp.tile([128, IT, 128], F16, tag="wd", name="wdn")
                    nc.sync.dma_start(w[:], dn[dt2])
                    ps = psum.tile([128, 512], F32, tag="mm", bufs=2, name="psd")
                    for it in range(IT):
                        nc.tensor.matmul(ps[:], w[:, it, :], act[:, it, :],
                                         start=(it == 0), stop=(it == IT - 1))
                    nc.vector.tensor_add(out=h[:, dt2, :], in0=h[:, dt2, :],
                                         in1=ps[:])
                _rmsnorm(h, hb)
                preload(AF.Exp)

            for lvl in level_calls:
                if lvl == "L":
                    h = zL
                    for dt in range(DT):
                        peng.tensor_tensor(h[:, dt, :], h[:, dt, :],
                                           zH[:, dt, :],
                                           mybir.AluOpType.add)
                        peng.tensor_tensor(h[:, dt, :], h[:, dt, :],
                                           emb[:, dt, :],
                                           mybir.AluOpType.add)
                    pre = "L"
                else:
                    h = zH
                    for dt in range(DT):
                        peng.tensor_tensor(h[:, dt, :], h[:, dt, :],
                                           zL[:, dt, :],
                                           mybir.AluOpType.add)
                    pre = "H"
                for dt in range(DT):
                    nc.vector.tensor_copy(out=hb[:, dt, :], in_=h[:, dt, :])
                for i in range(2):
                    block(
                        h,
                        inp[f"{pre}_wqT"][i], inp[f"{pre}_wkT"][i],
                        inp[f"{pre}_wvT"][i], inp[f"{pre}_woT"][i],
                        inp[f"{pre}_guT"][i], inp[f"{pre}_dnT"][i],
                    )

            nc.sync.dma_start(
                out_t.rearrange("(dt p) t -> p dt t", p=128), zH[:])

    nc.compile()
    return nc


def _prep_weights(inputs):
    bf = np.float16
    w = {}
    for pre in ("L", "H"):
        # [out, in] torch-style weights -> pre-tiled [L, ot, p(in), dt(in), m]
        for nm, src, mtile in [("wqT", "wq", 128), ("wkT", "wk", 128),
                               ("woT", "wo", 128)]:
            a = np.asarray(inputs[f"{pre}_{src}"])  # [2, D, D] = [l, o, i]
            t = a.reshape(2, DT, 128, DT, 128)       # [l, ot, m, dt, p]
            w[f"{pre}_{nm}"] = np.ascontiguousarray(
                t.transpose(0, 1, 4, 3, 2)).astype(bf)
        a = np.asarray(inputs[f"{pre}_wv"])          # [2, D, D]
        t = a.reshape(2, 4, 256, DT, 128)            # [l, oc, m, dt, p]
        w[f"{pre}_wvT"] = np.ascontiguousarray(
            t.transpose(0, 1, 4, 3, 2)).astype(bf)
        g = np.asarray(inputs[f"{pre}_gu"])          # [2, 2*INTER, D]
        gate = g[:, :INTER].reshape(2, IT, 128, DT, 128)
        up = g[:, INTER:].reshape(2, IT, 128, DT, 128)
        gu = np.concatenate([gate, up], axis=2)      # [l, it, 256(m), dt, p]
        w[f"{pre}_guT"] = np.ascontiguousarray(
            gu.transpose(0, 1, 4, 3, 2)).astype(bf)  # [l, it, p, dt, 256]
        d = np.asarray(inputs[f"{pre}_dn"])          # [2, D, INTER]
        t = d.reshape(2, DT, 128, IT, 128)           # [l, ot, m, it, p]
        w[f"{pre}_dnT"] = np.ascontiguousarray(
            t.transpose(0, 1, 4, 3, 2)).astype(bf)   # [l, ot, p, it, 128]
    cos = np.asarray(inputs["cos"])  # [S, 64]
    sin = np.asarray(inputs["sin"])
    cosT = np.tile(cos.T, (2, 1)).astype(np.float32)          # [128, S]
    sinT_s = sin.T.copy()
    sinT_s[:32] *= -1.0
    sinT = np.tile(sinT_s, (2, 1)).astype(np.float32)          # [128, S]
    return w, cosT, sinT


def kernel(**inputs):
    key = "nc"
    if key not in _CACHE:
        _CACHE[key] = build_kernel()
    nc = _CACHE[key]

    w, cosT, sinT = _prep_weights(inputs)
    zL = np.asarray(inputs["z_L"], np.float32)
    zH = np.asarray(inputs["z_H"], np.float32)
    emb = np.asarray(inputs["input_emb"], np.float32)

    in_maps = []
    for c in range(8):
        b, half = c // 2, c % 2
        sl = slice(half * T, (half + 1) * T)
        m = {
            "zL": np.ascontiguousarray(zL[b].T[:, sl]),
            "zH": np.ascontiguousarray(zH[b].T[:, sl]),
            "emb": np.ascontiguousarray(emb[b].T[:, sl]),
            "cosT": np.ascontiguousarray(cosT[:, sl]),
            "sinT": np.ascontiguousarray(sinT[:, sl]),
        }
        m.update(w)
        in_maps.append(m)

    trace = os.environ.get("HRM_TRACE", "0") == "1"
    res = run_bass_kernel_spmd(nc, in_maps, core_ids=list(range(8)), trace=trace)
    _CACHE["last_result"] = res

    out = np.empty((B, S, D), np.float32)
    for c in range(8):
        b, half = c // 2, c % 2
        out[b, half * T : (half + 1) * T, :] = res.results[c]["zH_out"].T
    return out


if __name__ == "__main__":
    rng = np.random.default_rng(0)
    ins = {
        "z_H": rng.standard_normal((B, S, D), np.float32),
        "z_L": rng.standard_normal((B, S, D), np.float32),
        "input_emb": rng.standard_normal((B, S, D), np.float32),
    }
    sd = 1.0 / np.sqrt(D)
    si = 1.0 / np.sqrt(INTER)
    for pre in ("L", "H"):
        for nm, shape, s in [("wq", (2, D, D), sd), ("wk", (2, D, D), sd),
                             ("wv", (2, D, D), sd), ("wo", (2, D, D), sd),
                             ("gu", (2, 2 * INTER, D), sd), ("dn", (2, D, INTER), si)]:
            ins[f"{pre}_{nm}"] = rng.standard_normal(shape, np.float32) * s
    inv = 1.0 / (10000.0 ** (np.arange(0, HD, 2, np.float32) / HD))
    fr = np.outer(np.arange(S, np.float32), inv)
    e = np.concatenate([fr, fr], -1)
    ins["cos"], ins["sin"] = np.cos(e).astype(np.float32), np.sin(e).astype(np.float32)
    out = kernel(**ins)
    print("out", out.shape, out.dtype, np.abs(out).mean())


# revision 12
# speedup vs baseline: 1.4392x; 1.2171x over previous
"""HRM dense-transformer kernel for 8 trn2 NeuronCores.

Sharding: data-parallel over batch (4) x sequence-parallel (2).
Core c handles batch b=c//2, token half h=c%2 (512 tokens).
Per block each core computes q/k/v for its own tokens, all-gathers
k^T and v (within its pair), then computes attention for its 512
queries over all 1024 keys. All activations are stored feature-major
([feature(part), token(free)]); scores are computed transposed [tk, tq]
so the softmax sum is a ones-matmul partition reduction. V is computed
token-major directly by swapping matmul operands, with a ones column
appended (M=65 matmul) so the softmax denominator falls out of the PV
matmul.

Perf structure (v2):
- scores are written into [128, 2, 512] 2-bank PSUM chunks and exp'd
  in [128, 1024] batches to amortize ACT per-op overhead.
- softmax normalize is deferred per head-pair: denominators collected
  to [2, 512], reciprocal via the fast custom-DVE approx, broadcast by
  ONE fp16 K=2 matmul (both heads at once) instead of fp32 K=1 pairs.
- rmsnorm uses ACT Square (in every act table -> no table reload) for
  h^2 and ACT Rsqrt for 1/sqrt(ms+eps); broadcast matmul in fp16.
- ACT table switches (Exp/Rsqrt/Silu) are prefetched with dummy [1,1]
  activations during PE-heavy phases.
- weights are pre-tiled host-side so every weight DMA reads 2KB
  contiguous per partition; gate+up column blocks are interleaved so
  one DMA feeds both.
"""

import os
import sys

sys.path.insert(0, "/opt/trn_rl_repo")

import ml_dtypes
import numpy as np

import concourse.bass as bass
import concourse.mybir as mybir
import concourse.tile as tile
from concourse import bacc
from concourse.bass_utils import run_bass_kernel_spmd

F32 = mybir.dt.float32
F16 = mybir.dt.float16
BF16 = mybir.dt.bfloat16
AF = mybir.ActivationFunctionType
MUL = mybir.AluOpType.mult

B, S, D, NH, HD = 4, 1024, 1024, 16, 64
INTER = 2816
T = S // 2              # own tokens per core
DT = D // 128           # 8 d-tiles
IT = INTER // 128       # 22 inter tiles
VF = NH * (HD + 1)      # 1040, v_aug feature width
EPS = 1e-5
KN = T * S              # kT elems (own): 1024 x 512
VN = T * VF             # v_aug elems (own): 512 x 1040
RG = [[0, 1], [2, 3], [4, 5], [6, 7]]

N_LEVEL_CALLS = int(os.environ.get("HRM_LEVEL_CALLS", "6"))
EXPSINGLE = os.environ.get("HRM_EXPSINGLE", "0") == "1"
NOPOOL = os.environ.get("HRM_NOPOOL", "0") == "1"
OLDNORM = os.environ.get("HRM_OLDNORM", "0") == "1"
OLDRMS = os.environ.get("HRM_OLDRMS", "0") == "1"

_CACHE = {}


def build_kernel():
    nc = bacc.Bacc("TRN2", target_bir_lowering=False, debug=False, num_devices=8)

    inp = {}
    for nm, shape, dt in [
        ("zL", [D, T], F32), ("zH", [D, T], F32), ("emb", [D, T], F32),
        ("cosT", [128, T], F32), ("sinT", [128, T], F32),
        # pre-tiled weights: [layer, out-tile, 128(p=in), in-tile, m]
        ("L_wqT", [2, DT, 128, DT, 128], F16),
        ("L_wkT", [2, DT, 128, DT, 128], F16),
        ("L_wvT", [2, 4, 128, DT, 256], F16),
        ("L_woT", [2, DT, 128, DT, 128], F16),
        ("L_guT", [2, IT, 128, DT, 256], F16),
        ("L_dnT", [2, DT, 128, IT, 128], F16),
        ("H_wqT", [2, DT, 128, DT, 128], F16),
        ("H_wkT", [2, DT, 128, DT, 128], F16),
        ("H_wvT", [2, 4, 128, DT, 256], F16),
        ("H_woT", [2, DT, 128, DT, 128], F16),
        ("H_guT", [2, IT, 128, DT, 256], F16),
        ("H_dnT", [2, DT, 128, IT, 128], F16),
    ]:
        inp[nm] = nc.dram_tensor(nm, shape, dt, kind="ExternalInput")
    out_t = nc.dram_tensor("zH_out", [D, T], F32, kind="ExternalOutput")

    seq = os.environ.get("HRM_SEQ", "")
    if seq:
        level_calls = list(seq)
    else:
        level_calls = (["L", "L", "H"] * 2)[:N_LEVEL_CALLS]

    with tile.TileContext(nc) as tc:
        with (
            tc.tile_pool(name="state", bufs=1) as st,
            tc.tile_pool(name="sp", bufs=2) as sp,
            tc.tile_pool(name="big", bufs=2) as bigp,
            tc.tile_pool(name="w128", bufs=4) as w128p,
            tc.tile_pool(name="w256", bufs=2) as w256p,
            tc.tile_pool(name="wd", bufs=2) as wdp,
            tc.tile_pool(name="pt", bufs=2) as ptp,
            tc.tile_pool(name="psum", bufs=1, space="PSUM") as psum,
            tc.tile_pool(name="dram", bufs=2, space="DRAM") as dram,
        ):
            zL = st.tile([128, DT, T], F32, name="zL_sb")
            zH = st.tile([128, DT, T], F32, name="zH_sb")
            emb = st.tile([128, DT, T], F32, name="emb_sb")
            cos = st.tile([128, T], F32, name="cos_sb")
            sin = st.tile([128, T], F32, name="sin_sb")
            hb = st.tile([128, DT, T], F16, name="hb")
            qT = st.tile([128, DT, T], F16, name="qT")
            kst = st.tile([128, DT, T], F16, name="kst")
            vst = st.tile([128, 4, VF], BF16, name="vst")
            oT = st.tile([128, DT, T], F16, name="oT")
            ones128 = st.tile([128, 1], F16, name="ones128")
            ones1f = st.tile([1, 128], F16, name="ones1f")
            sel64 = st.tile([64, 128], BF16, name="sel64")
            den2 = st.tile([64, 512], F32, name="den2")
            denr = st.tile([64, 512], F32, name="denr")
            denr16 = st.tile([64, 512], BF16, name="denr16")
            epsc = st.tile([1, 1], F32, name="epsc")
            dum = st.tile([1, 1], F32, name="dum")

            nc.sync.dma_start(zL[:], inp["zL"].rearrange("(dt p) t -> p dt t", p=128))
            nc.sync.dma_start(zH[:], inp["zH"].rearrange("(dt p) t -> p dt t", p=128))
            nc.sync.dma_start(emb[:], inp["emb"].rearrange("(dt p) t -> p dt t", p=128))
            nc.sync.dma_start(cos[:], inp["cosT"][:])
            nc.sync.dma_start(sin[:], inp["sinT"][:])
            nc.vector.memset(ones128[:], 1.0)
            nc.vector.memset(ones1f[:], 1.0)
            nc.vector.memset(epsc[:], EPS)
            nc.vector.memset(dum[:], 1.0)
            # selector for the softmax-denominator broadcast: row 0 feeds
            # out partitions 0-63 (even head), row 32 feeds 64-127 (odd).
            # den rows sit at partitions 0/32 (engine writes need 32-aligned
            # partition bases); unused rows stay at the 1.0 init so the
            # zero-weighted matmul columns never see NaN/Inf.
            nc.vector.memset(sel64[:], 0.0)
            nc.vector.memset(sel64[0:1, 0:64], 1.0)
            nc.vector.memset(sel64[32:33, 64:128], 1.0)
            nc.vector.memset(den2[:], 1.0)
            nc.vector.memset(denr[:], 1.0)
            nc.vector.memset(denr16[:], 1.0)
            # ones columns of v_aug (written once; data copies avoid them)
            nc.vector.memset(
                vst.rearrange("p tt (h c) -> p tt h c", c=HD + 1)[:, :, :, HD : HD + 1],
                1.0,
            )

            def preload(func):
                # dummy activation to pull the act table in early
                nc.scalar.activation(dum[:], dum[:], func)

            peng = nc.vector if NOPOOL else nc.gpsimd

            ones1f32 = st.tile([1, 128], F32, name="ones1f32")
            nc.vector.memset(ones1f32[:], 1.0)

            def _bcast_f32(row_f32):
                pb = psum.tile([128, 512], F32, tag="mm", bufs=2, name="pbf")
                nc.tensor.matmul(pb[:], ones1f32[:], row_f32, start=True,
                                 stop=True)
                bcf = sp.tile([128, 512], F32, tag="bcf", bufs=1, name="bcf")
                nc.vector.tensor_copy(out=bcf[:], in_=pb[:])
                return bcf

            def _rope(ps, out_ap):
                """out = ps*cos + rotate_half(ps)*sin.

                partition dim = 2 heads x 64. sinT rows are pre-negated for
                the first half of each head so a single multiply-add works.
                4 DVE ops: full-tile cos mult, two 2x32-partition sin mults
                (strided partition view pairs {0-31,64-95} <-> {32-63,96-127}),
                one add.
                """
                t1 = sp.tile([128, 512], F16, tag="rope1", name="rope1")
                t2 = sp.tile([128, 512], F16, tag="rope2", name="rope2")
                nc.vector.tensor_tensor(t1[:], ps[:], cos[:], MUL)
                nc.vector.tensor_tensor(t2[0:32, :], ps[32:64, :],
                                        sin[0:32, :], MUL)
                nc.vector.tensor_tensor(t2[32:64, :], ps[0:32, :],
                                        sin[32:64, :], MUL)
                nc.vector.tensor_tensor(t2[64:96, :], ps[96:128, :],
                                        sin[64:96, :], MUL)
                nc.vector.tensor_tensor(t2[96:128, :], ps[64:96, :],
                                        sin[96:128, :], MUL)
                nc.vector.tensor_add(out=out_ap, in0=t1[:], in1=t2[:])

            def _rmsnorm(h, hb):
                """h *= rsqrt(mean(h^2)+eps); hb = f16(h). ACT Square feeds a
                ones-matmul partition reduction; ACT Rsqrt (table prefetched)
                gives rstd in fp16; K=1 fp16 matmul broadcasts it."""
                sst = psum.tile([65, 512], F32, tag="pv", bufs=2, name="sst")
                ss = sst[0:1, :]
                for dt in range(DT):
                    r2 = sp.tile([128, 512], F16, tag="r2", name="r2")
                    nc.scalar.activation(r2[:], h[:, dt, :], AF.Square)
                    nc.tensor.matmul(ss, ones128[:], r2[:], start=(dt == 0),
                                     stop=(dt == DT - 1))
                sq = sp.tile([1, 512], F32, tag="sq", bufs=1, name="sq")
                nc.scalar.activation(sq[:], ss, AF.Sqrt, bias=epsc[:],
                                     scale=1.0 / D)
                if OLDRMS:
                    rstdf = sp.tile([1, 512], F32, tag="rstdf", bufs=1,
                                    name="rstdf")
                    nc.vector.reciprocal(rstdf[:], sq[:])
                    bcf = _bcast_f32(rstdf[:])
                    for dt in range(DT):
                        nc.vector.tensor_tensor(h[:, dt, :], h[:, dt, :],
                                                bcf[:], MUL)
                        nc.vector.tensor_copy(out=hb[:, dt, :],
                                              in_=h[:, dt, :])
                else:
                    rstdf = sp.tile([1, 512], F32, tag="rstdf", bufs=1,
                                    name="rstdf")
                    nc.vector.reciprocal_approx_fast(out=rstdf[:], in_=sq[:])
                    rstd = sp.tile([1, 512], F16, tag="rstd", bufs=1,
                                   name="rstd")
                    nc.vector.tensor_copy(out=rstd[:], in_=rstdf[:])
                    pb = psum.tile([128, 512], F32, tag="mm", bufs=2,
                                   name="pbn")
                    nc.tensor.matmul(pb[:], ones1f[:], rstd[:], start=True,
                                     stop=True)
                    bc = sp.tile([128, 512], F16, tag="bcn", name="bcn")
                    nc.vector.tensor_copy(out=bc[:], in_=pb[:])
                    for dt in range(DT):
                        nc.vector.tensor_tensor(hb[:, dt, :], h[:, dt, :],
                                                bc[:], MUL)
                        peng.tensor_tensor(h[:, dt, :], h[:, dt, :],
                                           bc[:], MUL)

            def block(h, wq, wk, wv, wo, gu, dn):
                gin_k = dram.tile([KN], F16, tag="gin_k", name="gin_k")
                gout_k = dram.tile([2 * KN], F16, tag="gout_k", name="gout_k")
                gin_v = dram.tile([VN], BF16, tag="gin_v", name="gin_v")
                gout_v = dram.tile([2 * VN], BF16, tag="gout_v", name="gout_v")

                # ---- k projection + rope ----
                for ot in range(DT):
                    w = w128p.tile([128, DT, 128], F16, tag="w128", name="wk")
                    nc.sync.dma_start(w[:], wk[ot])
                    ps = psum.tile([128, 512], F32, tag="mm", bufs=2, name="psk")
                    for dt in range(DT):
                        nc.tensor.matmul(ps[:], w[:, dt, :], hb[:, dt, :],
                                         start=(dt == 0), stop=(dt == DT - 1))
                    _rope(ps, kst[:, ot, :])
                # ---- send + gather k (overlaps v/q projection) ----
                nc.sync.dma_start(
                    gin_k[:].rearrange("(dt p t) -> p dt t", p=128, t=T), kst[:])
                nc.gpsimd.collective_compute(
                    "AllGather", mybir.AluOpType.bypass, replica_groups=RG,
                    ins=[gin_k.opt()], outs=[gout_k.opt()])
                # ---- v projection (token-major) ----
                vsr = vst.rearrange("p tt (hh c) -> p tt hh c", c=HD + 1)
                for oc in range(4):
                    w = w256p.tile([128, DT, 256], F16, tag="w256", name="wv")
                    nc.sync.dma_start(w[:], wv[oc])
                    for tt in range(4):
                        ps = psum.tile([128, 512], F32, tag="mm", bufs=2,
                                       name="psv")[:, 0:256]
                        for dt in range(DT):
                            nc.tensor.matmul(
                                ps[:], hb[:, dt, tt * 128 : (tt + 1) * 128],
                                w[:, dt, :], start=(dt == 0), stop=(dt == DT - 1))
                        nc.vector.tensor_copy(
                            out=vsr[:, tt, oc * 4 : (oc + 1) * 4, 0:HD],
                            in_=ps.rearrange("p (hh c) -> p hh c", c=HD))
                # ---- send + gather v (k already in flight) ----
                nc.sync.dma_start(
                    gin_v[:].rearrange("(tt p f) -> p tt f", p=128, f=VF), vst[:])
                nc.gpsimd.collective_compute(
                    "AllGather", mybir.AluOpType.bypass, replica_groups=RG,
                    ins=[gin_v.opt()], outs=[gout_v.opt()])
                # ---- q projection + rope (overlaps gather) ----
                for ot in range(DT):
                    w = w128p.tile([128, DT, 128], F16, tag="w128", name="wq")
                    nc.sync.dma_start(w[:], wq[ot])
                    ps = psum.tile([128, 512], F32, tag="mm", bufs=2, name="psq")
                    for dt in range(DT):
                        nc.tensor.matmul(ps[:], w[:, dt, :], hb[:, dt, :],
                                         start=(dt == 0), stop=(dt == DT - 1))
                    _rope(ps, qT[:, ot, :])
                # ---- load gathered k/v ----
                kTf = bigp.tile([128, DT, S], F16, tag="big", name="kTf")
                vf = bigp.tile([128, DT, VF], BF16, tag="big", name="vf")
                for r in range(2):
                    nc.sync.dma_start(
                        kTf[:, :, r * T : (r + 1) * T],
                        gout_k[r * KN : (r + 1) * KN].rearrange(
                            "(dt p t) -> p dt t", p=128, t=T))
                    nc.sync.dma_start(
                        vf[:, 4 * r : 4 * r + 4, :],
                        gout_v[r * VN : (r + 1) * VN].rearrange(
                            "(tt p f) -> p tt f", p=128, f=VF))
                # ---- attention ----
                # scores into [128,2,512] psum chunks, exp'd in [128,1024]
                # batches; PV accumulates per sub; normalize deferred per ot
                # with fast-recip + one fp16 K=2 broadcast matmul.
                for ot in range(DT):
                    pts = []
                    for sub in range(2):
                        bp = sub * 64
                        pt = ptp.tile([128, DT, 512], BF16, tag="pt", bufs=2,
                                      name="pt")
                        pts.append(pt)
                        for kc in range(4):
                            sc = psum.tile([128, 2, 512], F32, tag="sc", bufs=2,
                                           name="sc")
                            for j in range(2):
                                kt = kc * 2 + j
                                nc.tensor.matmul(
                                    sc[:, j, :],
                                    kTf[bp : bp + 64, ot, kt * 128 : (kt + 1) * 128],
                                    qT[bp : bp + 64, ot, :],
                                    start=True, stop=True, tile_position=(bp, 0))
                            if EXPSINGLE:
                                for j in range(2):
                                    nc.scalar.activation(
                                        pt[:, kc * 2 + j, :], sc[:, j, :],
                                        AF.Exp, scale=0.125)
                            else:
                                nc.scalar.activation(
                                    pt[:, kc * 2 : kc * 2 + 2, :], sc[:],
                                    AF.Exp, scale=0.125)
                    pvs = []
                    for sub in range(2):
                        hh = ot * 2 + sub
                        pv = psum.tile([65, 512], F32, tag="pv", bufs=2, name="pv")
                        for kt in range(DT):
                            nc.tensor.matmul(
                                pv[:],
                                vf[:, kt, hh * (HD + 1) : (hh + 1) * (HD + 1)],
                                pts[sub][:, kt, :],
                                start=(kt == 0), stop=(kt == DT - 1))
                        nc.vector.tensor_copy(out=den2[sub * 32 : sub * 32 + 1, :],
                                              in_=pv[64:65, :])
                        pvs.append(pv)
                    if ot == DT - 1:
                        preload(AF.Sqrt)
                    if OLDNORM:
                        for sub in range(2):
                            bp = sub * 64
                            recip = sp.tile([1, 512], F32, tag="recip",
                                            name="recip")
                            nc.vector.reciprocal(recip[:],
                                                 pvs[sub][64:65, :])
                            bcf = _bcast_f32(recip[:])
                            nc.vector.tensor_tensor(
                                oT[bp : bp + 64, ot, :], pvs[sub][0:HD, :],
                                bcf[bp : bp + 64, :], MUL)
                    else:
                        nc.vector.reciprocal_approx_fast(out=denr[:],
                                                         in_=den2[:])
                        nc.vector.tensor_copy(out=denr16[:], in_=denr[:])
                        pb = psum.tile([128, 512], F32, tag="mm", bufs=2,
                                       name="pbc")
                        nc.tensor.matmul(pb[:], sel64[:], denr16[:],
                                         start=True, stop=True)
                        bc = sp.tile([128, 512], BF16, tag="bc", name="bc")
                        nc.vector.tensor_copy(out=bc[:], in_=pb[:])
                        for sub in range(2):
                            bp = sub * 64
                            nc.vector.tensor_tensor(
                                oT[bp : bp + 64, ot, :], pvs[sub][0:HD, :],
                                bc[bp : bp + 64, :], MUL)
                # ---- o projection + residual ----
                for dt2 in range(DT):
                    w = w128p.tile([128, DT, 128], F16, tag="w128", name="wo")
                    nc.sync.dma_start(w[:], wo[dt2])
                    ps = psum.tile([128, 512], F32, tag="mm", bufs=2, name="pso")
                    for et in range(DT):
                        nc.tensor.matmul(ps[:], w[:, et, :], oT[:, et, :],
                                         start=(et == 0), stop=(et == DT - 1))
                    nc.vector.tensor_add(out=h[:, dt2, :], in0=h[:, dt2, :],
                                         in1=ps[:])
                _rmsnorm(h, hb)
                preload(AF.Silu)
                # ---- MLP ----
                act = bigp.tile([128, IT, 512], F16, tag="big", name="act")
                for it in range(IT):
                    wgu = w256p.tile([128, DT, 256], F16, tag="w256", name="wgu")
                    nc.sync.dma_start(wgu[:], gu[it])
                    sc = psum.tile([128, 2, 512], F32, tag="sc", bufs=2,
                                   name="scm")
                    for dt in range(DT):
                        nc.tensor.matmul(sc[:, 0, :], wgu[:, dt, 0:128],
                                         hb[:, dt, :],
                                         start=(dt == 0), stop=(dt == DT - 1))
                    for dt in range(DT):
                        nc.tensor.matmul(sc[:, 1, :], wgu[:, dt, 128:256],
                                         hb[:, dt, :],
                                         start=(dt == 0), stop=(dt == DT - 1))
                    sg = sp.tile([128, 512], F16, tag="sg", name="sg")
                    nc.scalar.activation(sg[:], sc[:, 0, :], AF.Silu)
                    nc.vector.tensor_tensor(act[:, it, :], sc[:, 1, :], sg[:], MUL)
                    if it == IT - 1:
                        preload(AF.Sqrt)
                for dt2 in range(DT):
                    w = wdp.tile([128, IT, 128], F16, tag="wd", name="wdn")
                    nc.sync.dma_start(w[:], dn[dt2])
                    ps = psum.tile([128, 512], F32, tag="mm", bufs=2, name="psd")
                    for it in range(IT):
                        nc.tensor.matmul(ps[:], w[:, it, :], act[:, it, :],
                                         start=(it == 0), stop=(it == IT - 1))
                    nc.vector.tensor_add(out=h[:, dt2, :], in0=h[:, dt2, :],
                                         in1=ps[:])
                _rmsnorm(h, hb)
                preload(AF.Exp)

            for lvl in level_calls:
                if lvl == "L":
                    h = zL
                    for dt in range(DT):
                        peng.tensor_tensor(h[:, dt, :], h[:, dt, :],
                                           zH[:, dt, :],
                                           mybir.AluOpType.add)
                        peng.tensor_tensor(h[:, dt, :], h[:, dt, :],
                                           emb[:, dt, :],
                                           mybir.AluOpType.add)
                    pre = "L"
                else:
                    h = zH
                    for dt in range(DT):
                        peng.tensor_tensor(h[:, dt, :], h[:, dt, :],
                                           zL[:, dt, :],
                                           mybir.AluOpType.add)
                    pre = "H"
                for dt in range(DT):
                    nc.vector.tensor_copy(out=hb[:, dt, :], in_=h[:, dt, :])
                for i in range(2):
                    block(
                        h,
                        inp[f"{pre}_wqT"][i], inp[f"{pre}_wkT"][i],
                        inp[f"{pre}_wvT"][i], inp[f"{pre}_woT"][i],
                        inp[f"{pre}_guT"][i], inp[f"{pre}_dnT"][i],
                    )

            nc.sync.dma_start(
                out_t.rearrange("(dt p) t -> p dt t", p=128), zH[:])

    nc.compile()
    return nc


def _prep_weights(inputs):
    bf = np.float16
    w = {}
    for pre in ("L", "H"):
        # [out, in] torch-style weights -> pre-tiled [L, ot, p(in), dt(in), m]
        for nm, src, mtile in [("wqT", "wq", 128), ("wkT", "wk", 128),
                               ("woT", "wo", 128)]:
            a = np.asarray(inputs[f"{pre}_{src}"])  # [2, D, D] = [l, o, i]
            t = a.reshape(2, DT, 128, DT, 128)       # [l, ot, m, dt, p]
            w[f"{pre}_{nm}"] = np.ascontiguousarray(
                t.transpose(0, 1, 4, 3, 2)).astype(bf)
        a = np.asarray(inputs[f"{pre}_wv"])          # [2, D, D]
        t = a.reshape(2, 4, 256, DT, 128)            # [l, oc, m, dt, p]
        w[f"{pre}_wvT"] = np.ascontiguousarray(
            t.transpose(0, 1, 4, 3, 2)).astype(bf)
        g = np.asarray(inputs[f"{pre}_gu"])          # [2, 2*INTER, D]
        gate = g[:, :INTER].reshape(2, IT, 128, DT, 128)
        up = g[:, INTER:].reshape(2, IT, 128, DT, 128)
        gu = np.concatenate([gate, up], axis=2)      # [l, it, 256(m), dt, p]
        w[f"{pre}_guT"] = np.ascontiguousarray(
            gu.transpose(0, 1, 4, 3, 2)).astype(bf)  # [l, it, p, dt, 256]
        d = np.asarray(inputs[f"{pre}_dn"])          # [2, D, INTER]
        t = d.reshape(2, DT, 128, IT, 128)           # [l, ot, m, it, p]
        w[f"{pre}_dnT"] = np.ascontiguousarray(
            t.transpose(0, 1, 4, 3, 2)).astype(bf)   # [l, ot, p, it, 128]
    cos = np.asarray(inputs["cos"])  # [S, 64]
    sin = np.asarray(inputs["sin"])
    cosT = np.tile(cos.T, (2, 1)).astype(np.float32)          # [128, S]
    sinT_s = sin.T.copy()
    sinT_s[:32] *= -1.0
    sinT = np.tile(sinT_s, (2, 1)).astype(np.float32)          # [128, S]
    return w, cosT, sinT


def kernel(**inputs):
    key = "nc"
    if key not in _CACHE:
        _CACHE[key] = build_kernel()
    nc = _CACHE[key]

    w, cosT, sinT = _prep_weights(inputs)
    zL = np.asarray(inputs["z_L"], np.float32)
    zH = np.asarray(inputs["z_H"], np.float32)
    emb = np.asarray(inputs["input_emb"], np.float32)

    in_maps = []
    for c in range(8):
        b, half = c // 2, c % 2
        sl = slice(half * T, (half + 1) * T)
        m = {
            "zL": np.ascontiguousarray(zL[b].T[:, sl]),
            "zH": np.ascontiguousarray(zH[b].T[:, sl]),
            "emb": np.ascontiguousarray(emb[b].T[:, sl]),
            "cosT": np.ascontiguousarray(cosT[:, sl]),
            "sinT": np.ascontiguousarray(sinT[:, sl]),
        }
        m.update(w)
        in_maps.append(m)

    trace = os.environ.get("HRM_TRACE", "0") == "1"
    res = run_bass_kernel_spmd(nc, in_maps, core_ids=list(range(8)), trace=trace)
    _CACHE["last_result"] = res

    out = np.empty((B, S, D), np.float32)
    for c in range(8):
        b, half = c // 2, c % 2
        out[b, half * T : (half + 1) * T, :] = res.results[c]["zH_out"].T
    return out


if __name__ == "__main__":
    rng = np.random.default_rng(0)
    ins = {
        "z_H": rng.standard_normal((B, S, D), np.float32),
        "z_L": rng.standard_normal((B, S, D), np.float32),
        "input_emb": rng.standard_normal((B, S, D), np.float32),
    }
    sd = 1.0 / np.sqrt(D)
    si = 1.0 / np.sqrt(INTER)
    for pre in ("L", "H"):
        for nm, shape, s in [("wq", (2, D, D), sd), ("wk", (2, D, D), sd),
                             ("wv", (2, D, D), sd), ("wo", (2, D, D), sd),
                             ("gu", (2, 2 * INTER, D), sd), ("dn", (2, D, INTER), si)]:
            ins[f"{pre}_{nm}"] = rng.standard_normal(shape, np.float32) * s
    inv = 1.0 / (10000.0 ** (np.arange(0, HD, 2, np.float32) / HD))
    fr = np.outer(np.arange(S, np.float32), inv)
    e = np.concatenate([fr, fr], -1)
    ins["cos"], ins["sin"] = np.cos(e).astype(np.float32), np.sin(e).astype(np.float32)
    out = kernel(**ins)
    print("out", out.shape, out.dtype, np.abs(out).mean())
